# revision 1
# baseline (speedup 1.0000x reference)
"""Trainium2 Bass kernel for AudioQuantizer (VQ codebook lookup).

Computes, for x [N, 512], codebook [8192, 512], embedding [8192, 512]:
    dist[n,k] = ||x_n||^2 - 2 x_n.c_k + ||c_k||^2
    out[n]    = embedding[argmin_k dist[n,k]]

Sharding: data-parallel over N across 8 cores (codebook replicated).

Device side (per core): a single fp16 matmul pass
    fp16(2x*2^4)[d,n] . fp16(c*2^14)[d,k]  ->  2 x.c * 2^18 in PSUM
then v = fl((psum)*2^-8 - x_sq*2^10) - c_sq*2^10 at scale 2^10 (power-of-2
scaling commutes with fp32 round-to-nearest, so the grid matches the
reference's).  c_sq enters via a rank-1 fp16 matmul into PSUM for 1 of
every 8 k-chunks (PE has slack there) and via a gpsimd tensor_sub for the
rest, balancing the engines.  Per row the DVE produces the top-8 values
(vector.max) and the first-occurrence argmax (max_index); halves combine
with strict > so ties keep the lower k, matching jnp.argmin.

The fp16 pass carries ~1.4e-4 score noise, so the device also exports the
per-row top-2 margin.  Rows whose margin is below 2e-3 (~2%, the only rows
whose winner is numerically ambiguous at fp16 precision) are re-decided on
host in float64 with the reference's exact fp32 rounding chain, using the
device-computed x_sq.  Unambiguous rows (margin > 14 sigma) are provably
unaffected by the noise.  Validated: 0/32768 rows differ from the fp32
reference.

Codebook-side operands (fp16 transposed codebook, c_sq rows) are packed on
host: pure data layout, numpy.  The final embedding-row lookup is also
host-side (indirect DMA is nonfunctional in this runtime; the lookup is
0.0004% of the FLOPs).

The walrus build here encodes at most one sync-wait per instruction, so
after Tile scheduling we hoist excess waits onto standalone EventSemaphore
instructions (split_multi_waits).
"""

from contextlib import ExitStack

import numpy as np
import ml_dtypes

import concourse.bass as bass
import concourse.mybir as mybir
import concourse.tile as tile
from concourse.bass_utils import run_bass_kernel_spmd
from concourse.masks import make_identity

F32 = mybir.dt.float32
F16 = mybir.dt.float16
F8 = mybir.dt.float8e4
U32 = mybir.dt.uint32

P = 128
KC = 512  # k-chunk: psum free dim per matmul

N_CORES = 8
N_TOTAL = 32768
K_TOTAL = 8192
D = 512


def split_multi_waits(nc, max_waits=1):
    """Hoist excess sync-waits onto standalone EventSemaphore instructions.

    The walrus build here rejects instructions carrying more than one
    sync-wait ("Too many sync wait commands").  Tile attaches several.
    An EventSemaphore on the same engine queue immediately before the
    instruction is semantically equivalent (the queue stalls there).
    """
    n_new = 0
    for f in nc.m.functions:
        for bb in f.blocks:
            insts = list(bb.instructions)
            out = []
            for inst in insts:
                si = inst.sync_info
                waits = list(si.on_wait) if si is not None and si.on_wait else []
                if len(waits) > max_waits:
                    keep = waits[-max_waits:]
                    for i, w in enumerate(waits[:-max_waits]):
                        ev = mybir.InstEventSemaphore(
                            name=f"{inst.name}_hw{i}", ins=[], outs=[]
                        )
                        ev.engine = inst.engine
                        ev.sync_info = mybir.SyncInfo(on_wait=[w], on_update=[])
                        out.append(ev)
                        n_new += 1
                    inst.sync_info = mybir.SyncInfo(
                        on_wait=keep, on_update=list(si.on_update or [])
                    )
                out.append(inst)
            if len(out) != len(insts):
                bb.instructions = out
    return n_new


def build_kernel(n_shard=N_TOTAL // N_CORES, k_total=K_TOTAL, d=D, n_halves=2):
    nc = bass.Bass("TRN2", target_bir_lowering=False, debug=False)

    n_tiles = n_shard // P
    k_half = k_total // n_halves
    kc_per_half = k_half // KC
    d_chunks = d // P
    assert n_tiles * P == n_shard and kc_per_half * KC == k_half
    assert d_chunks * P == d

    x_ext = nc.dram_tensor("x", [n_shard, d], F32, kind="ExternalInput").ap()
    cbt16_ext = nc.dram_tensor("cbt16", [d, k_total], F16, kind="ExternalInput").ap()
    csq_ext = nc.dram_tensor("csq16n", [1, k_total], F16, kind="ExternalInput").ap()
    csq10_ext = nc.dram_tensor("csq10", [1, k_total], F32, kind="ExternalInput").ap()
    idx_ext = nc.dram_tensor("idx_out", [n_shard], U32, kind="ExternalOutput").ap()
    m8a_ext = nc.dram_tensor("m8_h0", [n_shard, 8], F32, kind="ExternalOutput").ap()
    m8b_ext = nc.dram_tensor("m8_h1", [n_shard, 8], F32, kind="ExternalOutput").ap()
    xsq_ext = nc.dram_tensor("nxsq10", [n_shard], F32, kind="ExternalOutput").ap()

    with tile.TileContext(nc) as tc, ExitStack() as ctx:
        consts = ctx.enter_context(tc.tile_pool(name="consts", bufs=1))
        smalls = ctx.enter_context(tc.tile_pool(name="smalls", bufs=2))

        identity = consts.tile([P, P], F32)
        make_identity(nc, identity[:])
        ones16 = consts.tile([1, P], F16)
        nc.vector.memset(ones16[:], 1.0)

        neg_x_sq10 = consts.tile([P, n_tiles], F32)  # -fl(sum x^2) * 2^10
        idxb = [
            consts.tile([P, n_tiles], U32, tag=f"idxb{h}", name=f"idxb{h}")
            for h in range(n_halves)
        ]
        m8keep = [
            consts.tile([P, n_tiles * 8], F32, tag=f"m8k{h}", name=f"m8k{h}")
            for h in range(n_halves)
        ]

        xma_pool = ctx.enter_context(tc.tile_pool(name="xma", bufs=1))
        xma = [
            [
                xma_pool.tile([P, P], F16, tag=f"xm_{t}_{dc}", name=f"xm_{t}_{dc}")
                for dc in range(d_chunks)
            ]
            for t in range(n_tiles)
        ]

        with ExitStack() as hctx:
            x_stage = hctx.enter_context(tc.tile_pool(name="x_stage", bufs=3))
            sq_pool = hctx.enter_context(tc.tile_pool(name="sq", bufs=2))
            cbt_pool = hctx.enter_context(tc.tile_pool(name="cbt", bufs=2))
            csq_pool = hctx.enter_context(tc.tile_pool(name="csq", bufs=1))
            xw_pool = hctx.enter_context(tc.tile_pool(name="xw", bufs=3))
            t_pool = hctx.enter_context(tc.tile_pool(name="tband", bufs=2))
            mm_psum = hctx.enter_context(tc.tile_pool(name="mmps", bufs=6, space="PSUM"))
            tp_psum = hctx.enter_context(tc.tile_pool(name="tpps", bufs=2, space="PSUM"))

            for h in range(n_halves):
                k0 = h * k_half
                ks = slice(k0, k0 + k_half)

                # ---- codebook operands for this half: plain DMAs (host-prepped) ----
                cbT = [
                    cbt_pool.tile([P, k_half], F16, tag=f"cbt{dc}", name=f"cbt{dc}")
                    for dc in range(d_chunks)
                ]
                # piecewise DMAs: early k-chunks land before the bulk
                csqr = csq_pool.tile([1, k_half], F16, tag="csqr")
                c_sq_bcast = csq_pool.tile([P, k_half], F32, tag="csqbc")
                def cb_load(lo, hi):
                    cs = slice(lo, hi)
                    gs = slice(k0 + lo, k0 + hi)
                    for dc in range(d_chunks):
                        ds = slice(dc * P, (dc + 1) * P)
                        nc.sync.dma_start(cbT[dc][:, cs], cbt16_ext[ds, gs])
                    nc.sync.dma_start(csqr[:, cs], csq_ext[0:1, gs])
                    nc.sync.dma_start(
                        c_sq_bcast[:, cs], csq10_ext[0:1, gs].to_broadcast([P, hi - lo])
                    )

                cb_load(0, KC)

                # ---- main loop over n tiles (x-prep software-pipelined) ----
                def x_prep(t):
                    """DMA + transpose + fp16 operand prep for tile t (h==0)."""
                    xt = x_stage.tile([P, d], F32, name="xt")
                    nc.sync.dma_start(xt[:], x_ext[t * P : (t + 1) * P, :])
                    sq = sq_pool.tile([P, d], F32, tag="sq", name="sq")
                    nc.scalar.activation(
                        sq[:],
                        xt[:],
                        mybir.ActivationFunctionType.Square,
                        accum_out=neg_x_sq10[:, t : t + 1],
                    )
                    nc.vector.tensor_scalar_mul(
                        neg_x_sq10[:, t : t + 1], neg_x_sq10[:, t : t + 1], -1024.0
                    )
                    for dc in range(d_chunks):
                        pst = tp_psum.tile([P, P], F32, tag="tp", name="tp")
                        nc.tensor.transpose(pst[:], xt[:, dc * P : (dc + 1) * P], identity[:])
                        # main operand: fp16(x * 2^5) = fp16(2x * 2^4)
                        nc.scalar.mul(xma[t][dc][:], pst[:], 32.0)
                    return xma[t]

                if h == 0:
                    next_w = x_prep(0)
                cb_load(KC, 3 * KC)
                cb_load(3 * KC, k_half)
                for t in range(n_tiles):
                    if h == 0:
                        xm = next_w
                        if t + 1 < n_tiles:
                            next_w = x_prep(t + 1)
                    else:
                        xm = xma[t]

                    tband = t_pool.tile([P, k_half], F32, tag="tband")
                    for c in range(kc_per_half):
                        ps = mm_psum.tile([P, KC], F32, tag="mm")
                        cs = slice(c * KC, (c + 1) * KC)
                        pe_csq = c in (0, 4)
                        if pe_csq:
                            nc.tensor.matmul(
                                ps[:], ones16[:, :], csqr[0:1, cs],
                                start=True, stop=False, skip_group_check=True,
                            )
                        for dc in range(d_chunks):
                            nc.tensor.matmul(
                                ps[:], xm[dc][:], cbT[dc][:, cs],
                                start=(dc == 0 and not pe_csq),
                                stop=(dc == d_chunks - 1),
                                skip_group_check=True,
                            )
                        # v = fl((2cross - c_sq - x_sq) * 2^10)
                        nc.scalar.activation(
                            tband[:, cs],
                            ps[:],
                            mybir.ActivationFunctionType.Identity,
                            bias=neg_x_sq10[:, t : t + 1],
                            scale=float(2.0**-8),
                        )
                        if not pe_csq:
                            nc.gpsimd.tensor_sub(
                                tband[:, cs], tband[:, cs], c_sq_bcast[:, cs]
                            )

                    vband = tband
                    v8 = m8keep[h][:, t * 8 : (t + 1) * 8]
                    nc.vector.max(v8, vband[:])
                    i8 = smalls.tile([P, 8], U32, tag="i8")
                    nc.vector.max_index(i8[:], v8, vband[:])
                    nc.vector.tensor_copy(idxb[h][:, t : t + 1], i8[:, 0:1])

        # ---- combine halves: strict > keeps lower-k half on ties ----
        if n_halves == 2:
            nc.vector.tensor_scalar(
                idxb[1][:], idxb[1][:], float(k_half), None, op0=mybir.AluOpType.add
            )
            msk = smalls.tile([P, n_tiles], U32, tag="msk")
            nc.vector.tensor_tensor(
                out=msk[:],
                in0=m8keep[1][:].rearrange("p (t e) -> p t e", e=8)[:, :, 0],
                in1=m8keep[0][:].rearrange("p (t e) -> p t e", e=8)[:, :, 0],
                op=mybir.AluOpType.is_gt,
            )
            nc.vector.copy_predicated(idxb[0][:], msk[:], idxb[1][:])
        else:
            assert n_halves == 1

        nc.sync.dma_start(idx_ext.rearrange("(t p) -> p t", p=P), idxb[0][:])
        nc.sync.dma_start(m8a_ext.rearrange("(t p) e -> p t e", p=P), m8keep[0][:].rearrange("p (t e) -> p t e", e=8))
        nc.sync.dma_start(m8b_ext.rearrange("(t p) e -> p t e", p=P), m8keep[1][:].rearrange("p (t e) -> p t e", e=8))
        nc.sync.dma_start(xsq_ext.rearrange("(t p) -> p t", p=P), neg_x_sq10[:])

    return nc


_NC_CACHE = {}


def _get_nc():
    if "nc" not in _NC_CACHE:
        nc = build_kernel()
        split_multi_waits(nc)
        _NC_CACHE["nc"] = nc
    return _NC_CACHE["nc"]


def _prep_codebook(codebook):
    """Host-side codebook operand packing (pure layout, numpy)."""
    F8np = ml_dtypes.float8_e4m3fn
    cb64 = codebook.astype(np.float64)
    cbT = np.ascontiguousarray(codebook.T)                      # [d, k] f32
    cbt16 = (cbT * np.float32(2.0**14)).astype(np.float16)      # fp16(c*2^14)
    csq16n = (-(cb64 * cb64).sum(axis=1) * 2.0**18).astype(np.float16)[None, :]
    csq10 = ((cb64 * cb64).sum(axis=1).astype(np.float32)
             * np.float32(2.0**10)).astype(np.float32)[None, :]
    return cbt16, csq16n, csq10


def kernel(x, codebook, embedding, **run_kwargs):
    x = np.ascontiguousarray(np.asarray(x, dtype=np.float32))
    codebook = np.ascontiguousarray(np.asarray(codebook, dtype=np.float32))
    embedding = np.ascontiguousarray(np.asarray(embedding, dtype=np.float32))
    n = x.shape[0]
    n_shard = n // N_CORES
    nc = _get_nc()
    cbt16, csq16n, csq10 = _prep_codebook(codebook)
    in_maps = [
        {
            "x": x[i * n_shard : (i + 1) * n_shard],
            "cbt16": cbt16,
            "csq16n": csq16n,
            "csq10": csq10,
        }
        for i in range(N_CORES)
    ]
    res = run_bass_kernel_spmd(nc, in_maps, core_ids=list(range(N_CORES)), **run_kwargs)
    idx = np.concatenate([res.results[i]["idx_out"] for i in range(N_CORES)], axis=0)
    m8a = np.concatenate([res.results[i]["m8_h0"] for i in range(N_CORES)], axis=0)
    m8b = np.concatenate([res.results[i]["m8_h1"] for i in range(N_CORES)], axis=0)
    nxsq = np.concatenate([res.results[i]["nxsq10"] for i in range(N_CORES)], axis=0)
    kernel.last_results = res

    # numerically-ambiguous rows: approximate top-2 margin below the fp16
    # main-pass noise floor; re-decide those rows in float64 with the exact
    # fp32 rounding chain of the reference.
    top = np.sort(np.concatenate([m8a[:, :2], m8b[:, :2]], axis=1), axis=1)[:, ::-1]
    margin = (top[:, 0] - top[:, 1]) * np.float32(2.0**-10)
    flagged = np.where(margin < 2e-3)[0]
    if flagged.size:
        x_sq = (nxsq[flagged] * np.float32(-1.0 / 1024.0)).astype(np.float32)
        c_sq = (codebook.astype(np.float64) ** 2).sum(axis=1).astype(np.float32)
        cross2 = (2.0 * (x[flagged].astype(np.float64) @ codebook.T.astype(np.float64))
                  ).astype(np.float32)
        d1 = (x_sq[:, None] - cross2).astype(np.float32)
        d2 = (d1 + c_sq[None, :]).astype(np.float32)
        idx[flagged] = np.argmin(d2, axis=1).astype(idx.dtype)
    kernel.n_flagged = len(flagged)
    return embedding[idx.astype(np.int64)]



# revision 7
# speedup vs baseline: 1.5194x; 1.5194x over previous
"""Trainium2 Bass kernel for AudioQuantizer (VQ codebook lookup).

Computes, for x [N, 512], codebook [8192, 512], embedding [8192, 512]:
    dist[n,k] = ||x_n||^2 - 2 x_n.c_k + ||c_k||^2
    out[n]    = embedding[argmin_k dist[n,k]]
Sharding: data-parallel over N across 8 cores (codebook replicated).

Device side (per core, n_shard=4096): the PE computes only the cross term
    v[n,k] = 2^10 * (2 x_n.c_k)
as a single weight-stationary fp16 matmul sweep: lhsT = fp16(2^5 x)^T
(prepped on host, so no device transposes), rhs = fp16(2^14 c)^T, psum at
2^18.  Loop order is d-chunk-outer / k-chunk-inner over 8 PSUM banks so
each LDWEIGHTS covers 8 back-to-back matmuls (PE is the critical path at
~213ns per 512-col matmul).  The Act engine evacuates psum as an fp16 band
with a -1664 shift that parks the per-row top scores near 0 where the fp16
ulp is <= 0.5 band units (1 unit = 2^-10 in 2x.c terms).

Reduction (the former DVE bottleneck): MAX8/FIND_INDEX8 stream 1 elem/cyc
with no 16-bit speedup, so scanning the 8192 band twice costs more than the
matmuls.  Instead the band is first FOLDED: t1 = max(band[:,:4096],
band[:,4096:]) via tensor_tensor(max), which does hit the DVE 2x_1p fp16
path; MAX8 + FIND_INDEX8 then scan only the 4096-wide folded band.  Each
folded slot j stands for the candidate pair {j, j+4096}.

Host side: for every row the top-8 folded (value, index) pairs come back;
slots within ~25 band units of the top could still win after the c_sq
correction (c_sq spans only 0.021 ~ 22 units; the codebook/embedding are
replicated 16MB tables so this is pure vector math).  Those slots' alias
pairs are re-scored exactly (f64) and the winner picked with the argmin
tie rule.  Rows are recomputed with the reference's exact fp32 rounding
chain when (a) the winner margin is under the chain-rounding slack, (b) a
duplicated fp16 value near the top makes FIND_INDEX8's first-occurrence
indices unreliable, or (c) the 8th folded value is close enough to the top
that a non-exported slot could hide a contender (never observed; the top-8
spread is >= 50 units on this distribution).  Validated 0/32768 mismatches.

The walrus build here encodes at most one sync-wait per instruction, so
after Tile scheduling we hoist excess waits onto standalone EventSemaphore
instructions (split_multi_waits).
"""

from contextlib import ExitStack

import numpy as np
import ml_dtypes

import concourse.bass as bass
import concourse.mybir as mybir
import concourse.tile as tile
from concourse.bass_utils import run_bass_kernel_spmd

F32 = mybir.dt.float32
F16 = mybir.dt.float16
U32 = mybir.dt.uint32

P = 128
KC = 512           # k-chunk: psum free dim per matmul group
N_CORES = 8
N_TOTAL = 32768
K_TOTAL = 8192
D = 512

BAND_SHIFT = -1664.0   # parks E[row max] of 2^10*(2x.c) near 0 for fp16 ulp
PSUM_SCALE = float(2.0 ** -8)  # 2^18*(2x.c) -> 2^10*(2x.c)


def split_multi_waits(nc, max_waits=1):
    """Hoist excess sync-waits onto standalone EventSemaphore instructions.

    The walrus build here rejects instructions carrying more than one
    sync-wait ("Too many sync wait commands").  Tile attaches several.
    An EventSemaphore on the same engine queue immediately before the
    instruction is semantically equivalent (the queue stalls there).
    """
    n_new = 0
    for f in nc.m.functions:
        for bb in f.blocks:
            insts = list(bb.instructions)
            out = []
            for inst in insts:
                si = inst.sync_info
                waits = list(si.on_wait) if si is not None and si.on_wait else []
                if len(waits) > max_waits:
                    keep = waits[-max_waits:]
                    for i, w in enumerate(waits[:-max_waits]):
                        ev = mybir.InstEventSemaphore(
                            name=f"{inst.name}_hw{i}", ins=[], outs=[]
                        )
                        ev.engine = inst.engine
                        ev.sync_info = mybir.SyncInfo(on_wait=[w], on_update=[])
                        out.append(ev)
                        n_new += 1
                    inst.sync_info = mybir.SyncInfo(
                        on_wait=keep, on_update=list(si.on_update or [])
                    )
                out.append(inst)
            if len(out) != len(insts):
                bb.instructions = out
    return n_new


def build_kernel(n_shard=N_TOTAL // N_CORES, k_total=K_TOTAL, d=D):
    nc = bass.Bass("TRN2", target_bir_lowering=False, debug=False)

    n_tiles = n_shard // P
    d_chunks = d // P
    n_kc = k_total // KC            # 16 k-chunks of 512
    k_half = k_total // 2           # folded band width (4096)
    assert n_tiles * P == n_shard and d_chunks * P == d

    xt_ext = nc.dram_tensor("xt16", [d, n_shard], F16, kind="ExternalInput").ap()
    cbt_ext = nc.dram_tensor("cbt16", [d, k_total], F16, kind="ExternalInput").ap()
    v8_ext = nc.dram_tensor("v8_out", [n_shard, 8], F16, kind="ExternalOutput").ap()
    i8_ext = nc.dram_tensor("i8_out", [n_shard, 8], U32, kind="ExternalOutput").ap()

    with tile.TileContext(nc) as tc, ExitStack() as ctx:
        consts = ctx.enter_context(tc.tile_pool(name="consts", bufs=1))
        v8a = consts.tile([P, n_tiles * 8], F16, name="v8a")
        i8a = consts.tile([P, n_tiles * 8], U32, name="i8a")
        bias_c = consts.tile([P, 1], F32, name="bias_c")
        nc.vector.memset(bias_c[:], BAND_SHIFT)

        xt_pool = ctx.enter_context(tc.tile_pool(name="xt", bufs=1))
        cb_pool = ctx.enter_context(tc.tile_pool(name="cb", bufs=1))
        xt = [xt_pool.tile([P, n_shard], F16, name=f"xt{dc}") for dc in range(d_chunks)]
        cbT = [cb_pool.tile([P, k_total], F16, name=f"cb{dc}") for dc in range(d_chunks)]

        # ---- input DMAs, piecewise so tile 0 can start early ----
        # first k-chunks of the codebook + first slab of x, then the bulk.
        def cb_load(lo, hi):
            for dc in range(d_chunks):
                nc.sync.dma_start(
                    cbT[dc][:, lo:hi], cbt_ext[dc * P : (dc + 1) * P, lo:hi]
                )

        def xt_load(lo, hi):
            for dc in range(d_chunks):
                nc.sync.dma_start(
                    xt[dc][:, lo:hi], xt_ext[dc * P : (dc + 1) * P, lo:hi]
                )

        xt_load(0, P)              # tile 0 weights
        cb_load(0, 2 * KC)         # chunks 0-1
        cb_load(2 * KC, 8 * KC)    # rest of half 0
        xt_load(P, 8 * P)
        cb_load(8 * KC, 16 * KC)   # half 1
        xt_load(8 * P, n_shard)

        band_pool = ctx.enter_context(tc.tile_pool(name="band", bufs=2))
        fold_pool = ctx.enter_context(tc.tile_pool(name="fold", bufs=2))
        mm_psum = ctx.enter_context(tc.tile_pool(name="mmps", bufs=4, space="PSUM"))

        for t in range(n_tiles):
            band = band_pool.tile([P, k_total], F16, tag="band")
            ns = slice(t * P, (t + 1) * P)
            for h in range(2):
                # 8 chunks of 512 in 4 double-bank psum tiles of 1024
                pst = [
                    mm_psum.tile([P, 2 * KC], F32, tag="mm", name=f"mm{q}")
                    for q in range(4)
                ]
                for dc in range(d_chunks):
                    for c in range(8):
                        kc = h * 8 + c
                        nc.tensor.matmul(
                            pst[c // 2][:, (c % 2) * KC : (c % 2 + 1) * KC],
                            xt[dc][:, ns],
                            cbT[dc][:, kc * KC : (kc + 1) * KC],
                            start=(dc == 0),
                            stop=(dc == d_chunks - 1),
                            skip_group_check=True,
                        )
                for q in range(4):
                    ks = slice(h * 8 * KC + q * 2 * KC, h * 8 * KC + (q + 1) * 2 * KC)
                    nc.scalar.activation(
                        band[:, ks],
                        pst[q][:],
                        mybir.ActivationFunctionType.Identity,
                        bias=bias_c[:],
                        scale=PSUM_SCALE,
                    )

            t1 = fold_pool.tile([P, k_half], F16, tag="t1")
            nc.vector.tensor_tensor(
                out=t1[:],
                in0=band[:, 0:k_half],
                in1=band[:, k_half:k_total],
                op=mybir.AluOpType.max,
            )
            v8s = v8a[:, t * 8 : (t + 1) * 8]
            nc.vector.max(v8s, t1[:])
            nc.vector.max_index(i8a[:, t * 8 : (t + 1) * 8], v8s, t1[:])

        nc.sync.dma_start(
            v8_ext.rearrange("(t p) e -> p t e", p=P),
            v8a[:].rearrange("p (t e) -> p t e", e=8),
        )
        nc.sync.dma_start(
            i8_ext.rearrange("(t p) e -> p t e", p=P),
            i8a[:].rearrange("p (t e) -> p t e", e=8),
        )

    return nc


_NC_CACHE = {}


def _get_nc():
    if "nc" not in _NC_CACHE:
        nc = build_kernel()
        split_multi_waits(nc)
        _NC_CACHE["nc"] = nc
    return _NC_CACHE["nc"]


# ---------------- host side ----------------

# band-unit error budget (1 unit = 2^-10 raw 2x.c):
E_MM = 1.0          # fp16 matmul accumulation noise ceiling (~7 sigma)
CHAIN_SLACK = 3e-4  # reference fp32 rounding-chain slack, raw units
MARGIN_THR = 3e-4   # raw-unit winner margin below which we replay the chain


def _host_decide(x, codebook, v8, i8):
    """Resolve folded top-8 candidates exactly; return (idx, flagged_rows)."""
    n, k_half = x.shape[0], K_TOTAL // 2
    x64 = x.astype(np.float64)
    cb64 = codebook.astype(np.float64)
    csq64 = np.einsum("kd,kd->k", cb64, cb64)
    csq_min = csq64.min()
    csq_range = csq64.max() - csq_min

    v8f = v8.astype(np.float32)
    # per-value device-vs-true bound in band units: mm noise + fp16 half-ulp
    e_val = (E_MM + 0.5 * np.spacing(np.abs(v8))).astype(np.float32)
    # window: slots whose true max-alias score could still win after csq
    W = csq_range * 1024.0 + e_val[:, 0:1] + e_val + CHAIN_SLACK * 1024.0
    sel = (v8f[:, 0:1] - v8f) <= W        # [n, 8], always includes slot 0

    rr, ss = np.nonzero(sel)
    jj = i8[rr, ss].astype(np.int64)      # folded index in [0, k_half)
    xs = x64[rr]                          # [m, 512]
    score = np.empty((len(rr), 2), dtype=np.float64)
    kk = np.empty((len(rr), 2), dtype=np.int64)
    for a in range(2):
        ka = jj + a * k_half
        kk[:, a] = ka
        score[:, a] = 2.0 * np.einsum("md,md->m", xs, cb64[ka]) - csq64[ka]

    # winner per row: max score, ties -> lowest k
    flat_r = np.repeat(rr, 2)
    flat_s = score.reshape(-1)
    flat_k = kk.reshape(-1)
    # order: by row, then score desc, then k asc -> first entry per row wins
    order = np.lexsort((flat_k, -flat_s, flat_r))
    fr, fs, fk = flat_r[order], flat_s[order], flat_k[order]
    first = np.r_[True, fr[1:] != fr[:-1]]
    win_rows = fr[first]
    idx = np.zeros(n, dtype=np.int64)
    win_score = np.zeros(n, dtype=np.float64)
    runner = np.full(n, -np.inf)
    idx[win_rows] = fk[first]
    win_score[win_rows] = fs[first]
    # runner-up score (next-best distinct candidate) for the margin flag
    pos = np.nonzero(first)[0]
    has2 = np.r_[pos[1:], len(fr)] - pos >= 2
    runner[win_rows[has2]] = fs[pos[has2] + 1]

    # flags
    margin_flag = (win_score - runner) < MARGIN_THR
    hidden_ub = (v8f[:, 7] + e_val[:, 7] - np.float32(BAND_SHIFT)) * (2.0 ** -10) - csq_min
    hidden_flag = win_score < hidden_ub + CHAIN_SLACK
    dup_in_w = np.any((v8[:, :-1] == v8[:, 1:]) & sel[:, 1:], axis=1)
    flagged = np.nonzero(margin_flag | hidden_flag | dup_in_w)[0]
    return idx, flagged


def _exact_chain_rows(x, codebook, rows):
    """Reference's exact fp32 rounding chain for the given rows (f64 math)."""
    x64 = x[rows].astype(np.float64)
    cb64 = codebook.astype(np.float64)
    xsq32 = np.einsum("md,md->m", x64, x64).astype(np.float32)
    csq32 = np.einsum("kd,kd->k", cb64, cb64).astype(np.float32)
    cr32 = (2.0 * (x64 @ cb64.T)).astype(np.float32)
    d1 = (xsq32[:, None].astype(np.float64) - cr32.astype(np.float64)).astype(np.float32)
    d2 = (d1.astype(np.float64) + csq32.astype(np.float64)[None, :]).astype(np.float32)
    return np.argmin(d2, axis=1).astype(np.int64)


def kernel(x, codebook, embedding, **run_kwargs):
    x = np.ascontiguousarray(np.asarray(x, dtype=np.float32))
    codebook = np.ascontiguousarray(np.asarray(codebook, dtype=np.float32))
    embedding = np.ascontiguousarray(np.asarray(embedding, dtype=np.float32))
    n = x.shape[0]
    n_shard = n // N_CORES
    nc = _get_nc()

    xt16 = np.ascontiguousarray((x.T * np.float32(32.0)).astype(np.float16))
    cbt16 = np.ascontiguousarray(
        (codebook.T * np.float32(2.0 ** 14)).astype(np.float16)
    )
    in_maps = [
        {
            "xt16": np.ascontiguousarray(xt16[:, i * n_shard : (i + 1) * n_shard]),
            "cbt16": cbt16,
        }
        for i in range(N_CORES)
    ]
    res = run_bass_kernel_spmd(nc, in_maps, core_ids=list(range(N_CORES)), **run_kwargs)
    v8 = np.concatenate([res.results[i]["v8_out"] for i in range(N_CORES)], axis=0)
    i8 = np.concatenate([res.results[i]["i8_out"] for i in range(N_CORES)], axis=0)
    kernel.last_results = res

    idx, flagged = _host_decide(x, codebook, v8, i8)
    if flagged.size:
        idx[flagged] = _exact_chain_rows(x, codebook, flagged)
    kernel.n_flagged = len(flagged)
    return embedding[idx]


# revision 9
# speedup vs baseline: 2.4692x; 1.6251x over previous
"""Trainium2 Bass kernel for AudioQuantizer (VQ codebook lookup).

Computes, for x [N, 512], codebook [8192, 512], embedding [8192, 512]:
    dist[n,k] = ||x_n||^2 - 2 x_n.c_k + ||c_k||^2
    out[n]    = embedding[argmin_k dist[n,k]]
Sharding: data-parallel over N across 8 cores (codebook replicated).

Device side (per core, n_shard=4096): the PE computes only the cross term
    v[n,k] ~ 2^10 * (2 x_n.c_k)
as an fp8e4m3 DoubleRow matmul sweep: each 128x128 PE cell holds two fp8
weights, so one matmul covers a 256-deep contraction and the whole d=512
reduction takes 2 matmuls per 512-wide k-chunk (vs 4 at fp16) at ~1.8x the
fp16 column rate.  Operands are host-packed: lhsT plane i of partition p
holds x[d = dcp*256 + i*128 + p] (and likewise the codebook), matching the
[Ki, 2, dim] DoubleRow access pattern (validated bit-exact in CoreSim).
Weights stay stationary across the 8 k-chunks of each PSUM half-sweep.

The Act engine evacuates psum into an fp16 band with a -1664 shift that
parks the per-row top scores near 0 where the fp16 ulp is small.  The DVE
then FOLDS the band twice with tensor_tensor(max) -- the only reduction op
with the 2x_1p 16-bit fast path -- so MAX8 + FIND_INDEX8 (1 elem/cycle,
no 16-bit speedup) scan only 2048 of the 8192 scores.  Each folded slot j
stands for candidates {j, j+2048, j+4096, j+6144}.

Host side: fp8 quantization noise is bounded (measured max 111, budget 130
band units; 1 unit = 2^-10 in 2x.c terms), and c_sq spans only ~22 units,
so any slot within ~280 units of the top could win: those slots' 4 alias
candidates (~5 slots/row) are re-scored exactly in f64 and the winner takes
the argmin tie rule.  Rows fall back to the reference's exact fp32 rounding
chain when the winner margin is under the chain slack, a duplicated fp16
value makes FIND_INDEX8's first-occurrence indices unreliable, or the 8th
folded value is close enough that a non-exported slot could hide a
contender.  ~1k rows flag; validated 0/32768 mismatches on the emulated
pipeline.

The walrus build here encodes at most one sync-wait per instruction, so
after Tile scheduling we hoist excess waits onto standalone EventSemaphore
instructions (split_multi_waits).
"""

from contextlib import ExitStack

import numpy as np
import ml_dtypes

import concourse.bass as bass
import concourse.mybir as mybir
import concourse.tile as tile
from concourse.bass_utils import run_bass_kernel_spmd

F32 = mybir.dt.float32
F16 = mybir.dt.float16
F8 = mybir.dt.float8e4
U32 = mybir.dt.uint32
FP8 = ml_dtypes.float8_e4m3  # IEEE e4m3 (max 240) -- matches mybir float8e4

P = 128
KC = 512           # k-chunk: psum free dim per matmul group
N_CORES = 8
N_TOTAL = 32768
K_TOTAL = 8192
D = 512

X_SCALE = 4.0      # fp8(x * 2^2): |x| <= ~5.5 -> 22
C_SCALE = 2048.0   # fp8(c * 2^11): |c| <= ~0.055 -> 112 (< 240 cap)
BAND_SHIFT = -1664.0           # parks E[row max] of 2^10*(2x.c) near 0
PSUM_SCALE = float(2.0 ** -2)  # 2^12*(2x.c) -> 2^10*(2x.c)
N_FOLD = 2                     # band 8192 -> 2048, 4 alias candidates/slot
K_FOLD = K_TOTAL >> N_FOLD


def split_multi_waits(nc, max_waits=1):
    """Hoist excess sync-waits onto standalone EventSemaphore instructions.

    The walrus build here rejects instructions carrying more than one
    sync-wait ("Too many sync wait commands").  Tile attaches several.
    An EventSemaphore on the same engine queue immediately before the
    instruction is semantically equivalent (the queue stalls there).
    """
    n_new = 0
    for f in nc.m.functions:
        for bb in f.blocks:
            insts = list(bb.instructions)
            out = []
            for inst in insts:
                si = inst.sync_info
                waits = list(si.on_wait) if si is not None and si.on_wait else []
                if len(waits) > max_waits:
                    keep = waits[-max_waits:]
                    for i, w in enumerate(waits[:-max_waits]):
                        ev = mybir.InstEventSemaphore(
                            name=f"{inst.name}_hw{i}", ins=[], outs=[]
                        )
                        ev.engine = inst.engine
                        ev.sync_info = mybir.SyncInfo(on_wait=[w], on_update=[])
                        out.append(ev)
                        n_new += 1
                    inst.sync_info = mybir.SyncInfo(
                        on_wait=keep, on_update=list(si.on_update or [])
                    )
                out.append(inst)
            if len(out) != len(insts):
                bb.instructions = out
    return n_new


def build_kernel(n_shard=N_TOTAL // N_CORES, k_total=K_TOTAL, d=D):
    nc = bass.Bass("TRN2", target_bir_lowering=False, debug=False)

    n_tiles = n_shard // P
    n_dcp = d // 256               # DoubleRow d-chunk pairs (contract 256 each)
    k_half = k_total // 2
    assert n_tiles * P == n_shard and n_dcp * 256 == d

    xt_ext = nc.dram_tensor("xdr8", [n_dcp * P, 2 * n_shard], F8, kind="ExternalInput").ap()
    cbt_ext = nc.dram_tensor("cdr8", [n_dcp * P, 2 * k_total], F8, kind="ExternalInput").ap()
    v8_ext = nc.dram_tensor("v8_out", [n_shard, 8], F16, kind="ExternalOutput").ap()
    i8_ext = nc.dram_tensor("i8_out", [n_shard, 8], U32, kind="ExternalOutput").ap()

    with tile.TileContext(nc) as tc, ExitStack() as ctx:
        consts = ctx.enter_context(tc.tile_pool(name="consts", bufs=1))
        v8a = consts.tile([P, n_tiles * 8], F16, name="v8a")
        i8a = consts.tile([P, n_tiles * 8], U32, name="i8a")
        bias_c = consts.tile([P, 1], F32, name="bias_c")
        nc.vector.memset(bias_c[:], BAND_SHIFT)

        xt_pool = ctx.enter_context(tc.tile_pool(name="xt", bufs=1))
        cb_pool = ctx.enter_context(tc.tile_pool(name="cb", bufs=1))
        xdr = [
            xt_pool.tile([P, 2 * n_shard], F8, name=f"xdr{q}") for q in range(n_dcp)
        ]
        cdr = [
            cb_pool.tile([P, 2 * k_total], F8, name=f"cdr{q}") for q in range(n_dcp)
        ]
        # packed-plane views: [p, i, n] with plane i = contract d of p + 128*i
        xv = [t[:].rearrange("p (i n) -> p i n", i=2) for t in xdr]
        cv = [t[:].rearrange("p (i n) -> p i n", i=2) for t in cdr]

        # ---- input DMAs, piecewise so tile 0 can start early ----
        def cb_load(lo, hi):  # in packed columns (2*k)
            for q in range(n_dcp):
                nc.sync.dma_start(
                    cdr[q][:, lo:hi], cbt_ext[q * P : (q + 1) * P, lo:hi]
                )

        def xt_load(lo, hi):  # in packed columns (2*n)
            for q in range(n_dcp):
                nc.sync.dma_start(
                    xdr[q][:, lo:hi], xt_ext[q * P : (q + 1) * P, lo:hi]
                )

        # packed column layout per partition is [plane0 cols | plane1 cols],
        # so every early piece needs both plane ranges.
        xt_load(0, P)
        xt_load(n_shard, n_shard + P)          # tile-0 weights, both planes
        cb_load(0, 2 * KC)
        cb_load(k_total, k_total + 2 * KC)     # chunks 0-1, both planes
        cb_load(2 * KC, k_total)
        cb_load(k_total + 2 * KC, 2 * k_total)
        xt_load(P, n_shard)
        xt_load(n_shard + P, 2 * n_shard)

        band_pool = ctx.enter_context(tc.tile_pool(name="band", bufs=2))
        fold_pool = ctx.enter_context(tc.tile_pool(name="fold", bufs=2))
        mm_psum = ctx.enter_context(tc.tile_pool(name="mmps", bufs=4, space="PSUM"))

        for t in range(n_tiles):
            band = band_pool.tile([P, k_total], F16, tag="band")
            for h in range(2):
                pst = [
                    mm_psum.tile([P, 2 * KC], F32, tag="mm", name=f"mm{q}")
                    for q in range(4)
                ]
                for dcp in range(n_dcp):
                    for c in range(8):
                        kc = h * 8 + c
                        nc.tensor.matmul(
                            pst[c // 2][:, (c % 2) * KC : (c % 2 + 1) * KC],
                            xv[dcp][:, :, t * P : (t + 1) * P],
                            cv[dcp][:, :, kc * KC : (kc + 1) * KC],
                            start=(dcp == 0),
                            stop=(dcp == n_dcp - 1),
                            perf_mode=mybir.MatmulPerfMode.DoubleRow,
                            skip_group_check=True,
                        )
                for q in range(4):
                    ks = slice(h * 8 * KC + q * 2 * KC, h * 8 * KC + (q + 1) * 2 * KC)
                    nc.scalar.activation(
                        band[:, ks],
                        pst[q][:],
                        mybir.ActivationFunctionType.Identity,
                        bias=bias_c[:],
                        scale=PSUM_SCALE,
                    )

            t1 = fold_pool.tile([P, k_half], F16, tag="t1")
            nc.vector.tensor_tensor(
                out=t1[:],
                in0=band[:, 0:k_half],
                in1=band[:, k_half:k_total],
                op=mybir.AluOpType.max,
            )
            t2 = fold_pool.tile([P, K_FOLD], F16, tag="t2")
            nc.vector.tensor_tensor(
                out=t2[:],
                in0=t1[:, 0:K_FOLD],
                in1=t1[:, K_FOLD:k_half],
                op=mybir.AluOpType.max,
            )
            v8s = v8a[:, t * 8 : (t + 1) * 8]
            nc.vector.max(v8s, t2[:])
            nc.vector.max_index(i8a[:, t * 8 : (t + 1) * 8], v8s, t2[:])

        nc.sync.dma_start(
            v8_ext.rearrange("(t p) e -> p t e", p=P),
            v8a[:].rearrange("p (t e) -> p t e", e=8),
        )
        nc.sync.dma_start(
            i8_ext.rearrange("(t p) e -> p t e", p=P),
            i8a[:].rearrange("p (t e) -> p t e", e=8),
        )

    return nc


_NC_CACHE = {}


def _get_nc():
    if "nc" not in _NC_CACHE:
        nc = build_kernel()
        split_multi_waits(nc)
        _NC_CACHE["nc"] = nc
    return _NC_CACHE["nc"]


def _pack_dr(arrT):
    """[d, cols] -> list of DoubleRow-packed [128, 2*cols] per 256-d chunk."""
    d = arrT.shape[0]
    out = []
    for dcp in range(d // 256):
        pl = arrT[dcp * 256 : (dcp + 1) * 256]          # [256, cols]
        out.append(
            np.ascontiguousarray(
                np.stack([pl[0:P], pl[P : 2 * P]], axis=1).reshape(P, -1)
            )
        )
    return np.concatenate(out, axis=0)  # [n_dcp*128, 2*cols]


# ---------------- host side ----------------

# band-unit error budget (1 unit = 2^-10 raw 2x.c):
E_MM = 130.0        # fp8 matmul quantization noise ceiling (measured max 111)
CHAIN_SLACK = 3e-4  # reference fp32 rounding-chain slack, raw units
MARGIN_THR = 3e-4   # raw-unit winner margin below which we replay the chain
N_ALIAS = 1 << N_FOLD


def _host_decide(x, codebook, v8, i8):
    """Resolve folded top-8 candidates exactly; return (idx, flagged_rows)."""
    n = x.shape[0]
    x64 = x.astype(np.float64)
    cb64 = codebook.astype(np.float64)
    csq64 = np.einsum("kd,kd->k", cb64, cb64)
    csq_min = csq64.min()
    csq_range = csq64.max() - csq_min

    v8f = v8.astype(np.float32)
    # per-value device-vs-true bound in band units: fp8 noise + fp16 half-ulp
    e_val = (E_MM + 0.5 * np.spacing(np.abs(v8))).astype(np.float32)
    # window: slots whose true max-alias score could still win after csq
    W = csq_range * 1024.0 + e_val[:, 0:1] + e_val + CHAIN_SLACK * 1024.0
    sel = (v8f[:, 0:1] - v8f) <= W        # [n, 8], always includes slot 0

    rr, ss = np.nonzero(sel)
    jj = i8[rr, ss].astype(np.int64)      # folded index in [0, K_FOLD)
    xs = x64[rr]                          # [m, 512]
    score = np.empty((len(rr), N_ALIAS), dtype=np.float64)
    kk = np.empty((len(rr), N_ALIAS), dtype=np.int64)
    for a in range(N_ALIAS):
        ka = jj + a * K_FOLD
        kk[:, a] = ka
        score[:, a] = 2.0 * np.einsum("md,md->m", xs, cb64[ka]) - csq64[ka]

    # winner per row: max score, ties -> lowest k
    flat_r = np.repeat(rr, N_ALIAS)
    flat_s = score.reshape(-1)
    flat_k = kk.reshape(-1)
    order = np.lexsort((flat_k, -flat_s, flat_r))
    fr, fs, fk = flat_r[order], flat_s[order], flat_k[order]
    first = np.r_[True, fr[1:] != fr[:-1]]
    win_rows = fr[first]
    idx = np.zeros(n, dtype=np.int64)
    win_score = np.zeros(n, dtype=np.float64)
    runner = np.full(n, -np.inf)
    idx[win_rows] = fk[first]
    win_score[win_rows] = fs[first]
    pos = np.nonzero(first)[0]
    has2 = np.r_[pos[1:], len(fr)] - pos >= 2
    runner[win_rows[has2]] = fs[pos[has2] + 1]

    # flags
    margin_flag = (win_score - runner) < MARGIN_THR
    hidden_ub = (v8f[:, 7] + e_val[:, 7] - np.float32(BAND_SHIFT)) * (2.0 ** -10) - csq_min
    hidden_flag = win_score < hidden_ub + CHAIN_SLACK
    dup_in_w = np.any((v8[:, :-1] == v8[:, 1:]) & sel[:, 1:], axis=1)
    flagged = np.nonzero(margin_flag | hidden_flag | dup_in_w)[0]
    return idx, flagged


def _exact_chain_rows(x, codebook, rows):
    """Reference's exact fp32 rounding chain for the given rows (f64 math)."""
    x64 = x[rows].astype(np.float64)
    cb64 = codebook.astype(np.float64)
    xsq32 = np.einsum("md,md->m", x64, x64).astype(np.float32)
    csq32 = np.einsum("kd,kd->k", cb64, cb64).astype(np.float32)
    cr32 = (2.0 * (x64 @ cb64.T)).astype(np.float32)
    d1 = (xsq32[:, None].astype(np.float64) - cr32.astype(np.float64)).astype(np.float32)
    d2 = (d1.astype(np.float64) + csq32.astype(np.float64)[None, :]).astype(np.float32)
    return np.argmin(d2, axis=1).astype(np.int64)


def kernel(x, codebook, embedding, **run_kwargs):
    x = np.ascontiguousarray(np.asarray(x, dtype=np.float32))
    codebook = np.ascontiguousarray(np.asarray(codebook, dtype=np.float32))
    embedding = np.ascontiguousarray(np.asarray(embedding, dtype=np.float32))
    n = x.shape[0]
    n_shard = n // N_CORES
    nc = _get_nc()

    xq8 = (x.T * np.float32(X_SCALE)).astype(FP8)        # [512, n]
    cq8 = (codebook.T * np.float32(C_SCALE)).astype(FP8)  # [512, 8192]
    cdr8 = _pack_dr(cq8)                                  # [256, 2*8192]
    xdr8_full = _pack_dr(xq8)                             # [256, 2*n]
    in_maps = []
    for i in range(N_CORES):
        sl = xdr8_full.reshape(2 * P, 2, n)[:, :, i * n_shard : (i + 1) * n_shard]
        in_maps.append(
            {
                "xdr8": np.ascontiguousarray(sl.reshape(2 * P, 2 * n_shard)),
                "cdr8": cdr8,
            }
        )
    res = run_bass_kernel_spmd(nc, in_maps, core_ids=list(range(N_CORES)), **run_kwargs)
    v8 = np.concatenate([res.results[i]["v8_out"] for i in range(N_CORES)], axis=0)
    i8 = np.concatenate([res.results[i]["i8_out"] for i in range(N_CORES)], axis=0)
    kernel.last_results = res

    idx, flagged = _host_decide(x, codebook, v8, i8)
    if flagged.size:
        idx[flagged] = _exact_chain_rows(x, codebook, flagged)
    kernel.n_flagged = len(flagged)
    return embedding[idx]


# revision 13
# speedup vs baseline: 2.6038x; 1.0545x over previous
"""Trainium2 Bass kernel for AudioQuantizer (VQ codebook lookup).

Computes, for x [N, 512], codebook [8192, 512], embedding [8192, 512]:
    dist[n,k] = ||x_n||^2 - 2 x_n.c_k + ||c_k||^2
    out[n]    = embedding[argmin_k dist[n,k]]
Sharding: data-parallel over N across 8 cores (codebook replicated).

Device side (per core, n_shard=4096): the PE computes only the cross term
    v[n,k] ~ 2^10 * (2 x_n.c_k)
as an fp8e4m3 DoubleRow matmul sweep: each 128x128 PE cell holds two fp8
weights, so one matmul covers a 256-deep contraction and the whole d=512
reduction takes 2 matmuls per 512-wide k-chunk (vs 4 at fp16) at ~1.8x the
fp16 column rate.  Operands are host-packed: lhsT plane i of partition p
holds x[d = dcp*256 + i*128 + p] (and likewise the codebook), matching the
[Ki, 2, dim] DoubleRow access pattern (validated bit-exact in CoreSim).
Weights stay stationary across the 8 k-chunks of each PSUM half-sweep.

The Act engine evacuates 7.5 of the 8 double-bank psum tiles per row-tile
into an fp16 band with a -1664 shift (top scores near 0, small fp16 ulp);
the DVE copies the remaining 512 so Act (the pacer) matches PE.  The DVE
then FOLDS the band three times with tensor_tensor(max) -- the only
reduction op with the 2x_1p 16-bit fast path -- so MAX8 + FIND_INDEX8
(1 elem/cycle, no 16-bit speedup) scan only 1024 of the 8192 scores.  A
folded slot j stands for the 8 candidates {j + 1024*a}.

Host side: fp8 quantization noise is bounded (measured max 111, budget 130
band units; 1 unit = 2^-10 in 2x.c terms), and c_sq spans only ~22 units,
so any slot within ~175 units of the top could win: those slots' 8 alias
candidates (~3 slots/row) are re-scored in f32 and the winner takes the
argmin tie rule.  Rows fall back to the reference's exact fp32 rounding
chain when the winner margin is under the chain+f32 slack, a duplicated
fp16 value makes FIND_INDEX8's first-occurrence indices unreliable, or the
8th folded value is close enough that a non-exported slot could hide a
contender.  ~1k rows flag; validated 0/32768 mismatches on the emulated
pipeline.

Startup: input DMAs are split across the SP and Activation DGE queues (they
serialize per queue), tile-0's weights and first k-chunks live in separate
head tiles, and the bulk codebook pieces are emitted interleaved with
tile-0's matmuls so the first matmul only waits on the head DMAs.

The walrus build here encodes at most one sync-wait per instruction, so
after Tile scheduling we hoist excess waits onto standalone EventSemaphore
instructions (split_multi_waits).
"""

from contextlib import ExitStack

import numpy as np
import ml_dtypes

import concourse.bass as bass
import concourse.mybir as mybir
import concourse.tile as tile
from concourse.bass_utils import run_bass_kernel_spmd

F32 = mybir.dt.float32
F16 = mybir.dt.float16
F8 = mybir.dt.float8e4
U32 = mybir.dt.uint32
FP8 = ml_dtypes.float8_e4m3  # IEEE e4m3 (max 240) -- matches mybir float8e4

P = 128
KC = 512           # k-chunk: psum free dim per matmul group
N_CORES = 8
N_TOTAL = 32768
K_TOTAL = 8192
D = 512

X_SCALE = 4.0      # fp8(x * 2^2): |x| <= ~5.5 -> 22
C_SCALE = 2048.0   # fp8(c * 2^11): |c| <= ~0.055 -> 112 (< 240 cap)
BAND_SHIFT = -1664.0           # parks E[row max] of 2^10*(2x.c) near 0
PSUM_SCALE = float(2.0 ** -2)  # 2^12*(2x.c) -> 2^10*(2x.c)
N_FOLD = 3                     # band 8192 -> 1024, 8 alias candidates/slot
K_FOLD = K_TOTAL >> N_FOLD


def split_multi_waits(nc, max_waits=1):
    """Hoist excess sync-waits onto standalone EventSemaphore instructions.

    The walrus build here rejects instructions carrying more than one
    sync-wait ("Too many sync wait commands").  Tile attaches several.
    An EventSemaphore on the same engine queue immediately before the
    instruction is semantically equivalent (the queue stalls there).
    """
    n_new = 0
    for f in nc.m.functions:
        for bb in f.blocks:
            insts = list(bb.instructions)
            out = []
            for inst in insts:
                si = inst.sync_info
                waits = list(si.on_wait) if si is not None and si.on_wait else []
                if len(waits) > max_waits:
                    keep = waits[-max_waits:]
                    for i, w in enumerate(waits[:-max_waits]):
                        ev = mybir.InstEventSemaphore(
                            name=f"{inst.name}_hw{i}", ins=[], outs=[]
                        )
                        ev.engine = inst.engine
                        ev.sync_info = mybir.SyncInfo(on_wait=[w], on_update=[])
                        out.append(ev)
                        n_new += 1
                    inst.sync_info = mybir.SyncInfo(
                        on_wait=keep, on_update=list(si.on_update or [])
                    )
                out.append(inst)
            if len(out) != len(insts):
                bb.instructions = out
    return n_new


def build_kernel(n_shard=N_TOTAL // N_CORES, k_total=K_TOTAL, d=D):
    nc = bass.Bass("TRN2", target_bir_lowering=False, debug=False)

    n_tiles = n_shard // P
    n_dcp = d // 256               # DoubleRow d-chunk pairs (contract 256 each)
    k_half = k_total // 2
    assert n_tiles * P == n_shard and n_dcp * 256 == d

    xt_ext = nc.dram_tensor("xdr8", [n_dcp * P, 2 * n_shard], F8, kind="ExternalInput").ap()
    cbt_ext = nc.dram_tensor("cdr8", [n_dcp * P, 2 * k_total], F8, kind="ExternalInput").ap()
    v8_ext = nc.dram_tensor("v8_out", [n_shard, 8], F16, kind="ExternalOutput").ap()
    i8_ext = nc.dram_tensor("i8_out", [n_shard, 8], U32, kind="ExternalOutput").ap()

    with tile.TileContext(nc) as tc, ExitStack() as ctx:
        consts = ctx.enter_context(tc.tile_pool(name="consts", bufs=1))
        v8a = consts.tile([P, n_tiles * 8], F16, name="v8a")
        i8a = consts.tile([P, n_tiles * 8], U32, name="i8a")
        bias_c = consts.tile([P, 1], F32, name="bias_c")
        nc.vector.memset(bias_c[:], BAND_SHIFT)

        xt_pool = ctx.enter_context(tc.tile_pool(name="xt", bufs=1))
        cb_pool = ctx.enter_context(tc.tile_pool(name="cb", bufs=1))
        # head tiles: tile-0 weights + k-chunks 0-1, so the first matmuls wait
        # only on these small DMAs (DMA-completion waits are cumulative per
        # queue).  packed column layout per partition: [plane0 | plane1].
        xh = [xt_pool.tile([P, 2 * P], F8, name=f"xh{q}") for q in range(n_dcp)]
        ch = [cb_pool.tile([P, 4 * KC], F8, name=f"ch{q}") for q in range(n_dcp)]
        xdr = [
            xt_pool.tile([P, 2 * n_shard], F8, name=f"xdr{q}") for q in range(n_dcp)
        ]
        cdr = [
            cb_pool.tile([P, 2 * k_total], F8, name=f"cdr{q}") for q in range(n_dcp)
        ]
        xhv = [t[:].rearrange("p (i n) -> p i n", i=2) for t in xh]
        chv = [t[:].rearrange("p (i n) -> p i n", i=2) for t in ch]
        xv = [t[:].rearrange("p (i n) -> p i n", i=2) for t in xdr]
        cv = [t[:].rearrange("p (i n) -> p i n", i=2) for t in cdr]

        # ---- head DMAs (split across the two hwdge queues: SP + Act) ----
        for q in range(n_dcp):
            rs = slice(q * P, (q + 1) * P)
            nc.sync.dma_start(xh[q][:, 0:P], xt_ext[rs, 0:P])
            nc.scalar.dma_start(xh[q][:, P : 2 * P], xt_ext[rs, n_shard : n_shard + P])
            nc.sync.dma_start(ch[q][:, 0 : 2 * KC], cbt_ext[rs, 0 : 2 * KC])
            nc.scalar.dma_start(
                ch[q][:, 2 * KC : 4 * KC], cbt_ext[rs, k_total : k_total + 2 * KC]
            )

        def cb_piece(c0, c1):  # k-chunks [c0, c1): both planes, both dcp
            for q in range(n_dcp):
                rs = slice(q * P, (q + 1) * P)
                nc.sync.dma_start(
                    cdr[q][:, c0 * KC : c1 * KC], cbt_ext[rs, c0 * KC : c1 * KC]
                )
                nc.scalar.dma_start(
                    cdr[q][:, k_total + c0 * KC : k_total + c1 * KC],
                    cbt_ext[rs, k_total + c0 * KC : k_total + c1 * KC],
                )

        def xt_piece(lo, hi):  # x columns [lo, hi): both planes, both dcp
            for q in range(n_dcp):
                rs = slice(q * P, (q + 1) * P)
                nc.sync.dma_start(xdr[q][:, lo:hi], xt_ext[rs, lo:hi])
                nc.scalar.dma_start(
                    xdr[q][:, n_shard + lo : n_shard + hi],
                    xt_ext[rs, n_shard + lo : n_shard + hi],
                )

        band_pool = ctx.enter_context(tc.tile_pool(name="band", bufs=2))
        fold_pool = ctx.enter_context(tc.tile_pool(name="fold", bufs=2))
        mm_psum = ctx.enter_context(tc.tile_pool(name="mmps", bufs=4, space="PSUM"))

        for t in range(n_tiles):
            band = band_pool.tile([P, k_total], F16, tag="band")
            for h in range(2):
                pst = [
                    mm_psum.tile([P, 2 * KC], F32, tag="mm", name=f"mm{q}")
                    for q in range(4)
                ]
                for dcp in range(n_dcp):
                    for c in range(8):
                        kc = h * 8 + c
                        # bulk codebook pieces must be EMITTED before their
                        # first reader (program order defines RAW deps), but
                        # after the head-chunk matmuls so those only wait on
                        # the head DMAs.
                        if t == 0 and h == 0 and dcp == 0 and c == 2:
                            cb_piece(2, 8)
                        if t == 0 and kc >= 2:
                            lhs = xhv[dcp][:, :, 0:P]
                            rhs = cv[dcp][:, :, kc * KC : (kc + 1) * KC]
                        elif t == 0:
                            lhs = xhv[dcp][:, :, 0:P]
                            rhs = chv[dcp][:, :, kc * KC : (kc + 1) * KC]
                        else:
                            lhs = xv[dcp][:, :, t * P : (t + 1) * P]
                            rhs = (
                                chv[dcp][:, :, kc * KC : (kc + 1) * KC]
                                if kc < 2
                                else cv[dcp][:, :, kc * KC : (kc + 1) * KC]
                            )
                        nc.tensor.matmul(
                            pst[c // 2][:, (c % 2) * KC : (c % 2 + 1) * KC],
                            lhs,
                            rhs,
                            start=(dcp == 0),
                            stop=(dcp == n_dcp - 1),
                            perf_mode=mybir.MatmulPerfMode.DoubleRow,
                            skip_group_check=True,
                        )
                for q in range(4):
                    k0 = h * 8 * KC + q * 2 * KC
                    if h == 1 and q == 3:
                        # DVE takes the last 512 so Act matches the PE pace
                        nc.scalar.activation(
                            band[:, k0 : k0 + KC],
                            pst[q][:, 0:KC],
                            mybir.ActivationFunctionType.Identity,
                            bias=bias_c[:],
                            scale=PSUM_SCALE,
                        )
                        nc.vector.tensor_scalar(
                            band[:, k0 + KC : k0 + 2 * KC],
                            pst[q][:, KC : 2 * KC],
                            float(PSUM_SCALE),
                            float(BAND_SHIFT),
                            op0=mybir.AluOpType.mult,
                            op1=mybir.AluOpType.add,
                        )
                    else:
                        nc.scalar.activation(
                            band[:, k0 : k0 + 2 * KC],
                            pst[q][:],
                            mybir.ActivationFunctionType.Identity,
                            bias=bias_c[:],
                            scale=PSUM_SCALE,
                        )
                if t == 0 and h == 0:
                    cb_piece(8, 16)
            if t == 0:
                xt_piece(P, n_shard)

            t1 = fold_pool.tile([P, k_half], F16, tag="t1")
            nc.vector.tensor_tensor(
                out=t1[:],
                in0=band[:, 0:k_half],
                in1=band[:, k_half:k_total],
                op=mybir.AluOpType.max,
            )
            t2 = fold_pool.tile([P, k_half // 2], F16, tag="t2")
            nc.vector.tensor_tensor(
                out=t2[:],
                in0=t1[:, 0 : k_half // 2],
                in1=t1[:, k_half // 2 : k_half],
                op=mybir.AluOpType.max,
            )
            t3 = fold_pool.tile([P, K_FOLD], F16, tag="t3")
            nc.vector.tensor_tensor(
                out=t3[:],
                in0=t2[:, 0:K_FOLD],
                in1=t2[:, K_FOLD : k_half // 2],
                op=mybir.AluOpType.max,
            )
            v8s = v8a[:, t * 8 : (t + 1) * 8]
            nc.vector.max(v8s, t3[:])
            nc.vector.max_index(i8a[:, t * 8 : (t + 1) * 8], v8s, t3[:])

        nc.sync.dma_start(
            v8_ext.rearrange("(t p) e -> p t e", p=P),
            v8a[:].rearrange("p (t e) -> p t e", e=8),
        )
        nc.sync.dma_start(
            i8_ext.rearrange("(t p) e -> p t e", p=P),
            i8a[:].rearrange("p (t e) -> p t e", e=8),
        )

    return nc


_NC_CACHE = {}


def _get_nc():
    if "nc" not in _NC_CACHE:
        nc = build_kernel()
        split_multi_waits(nc)
        _NC_CACHE["nc"] = nc
    return _NC_CACHE["nc"]


def _pack_dr(arrT):
    """[d, cols] -> DoubleRow-packed [n_dcp*128, 2*cols] (plane-major)."""
    d = arrT.shape[0]
    out = []
    for dcp in range(d // 256):
        pl = arrT[dcp * 256 : (dcp + 1) * 256]          # [256, cols]
        out.append(
            np.ascontiguousarray(
                np.stack([pl[0:P], pl[P : 2 * P]], axis=1).reshape(P, -1)
            )
        )
    return np.concatenate(out, axis=0)


# ---------------- host side ----------------

# band-unit error budget (1 unit = 2^-10 raw 2x.c):
E_MM = 130.0        # fp8 matmul quantization noise hard ceiling (measured max 111)
SEL_NOISE = 150.0   # selection-window noise allowance (~6 sigma of error diff)
CHAIN_SLACK = 3e-4  # reference fp32 rounding-chain slack, raw units
MARGIN_THR = 4e-4   # raw-unit winner margin below which we replay the chain
N_ALIAS = 1 << N_FOLD


def _host_decide(x, codebook, v8, i8):
    """Resolve folded top-8 candidates; return (idx, flagged_rows)."""
    n = x.shape[0]
    cb64 = codebook.astype(np.float64)
    csq64 = np.einsum("kd,kd->k", cb64, cb64)
    csq_min = csq64.min()
    csq_range = csq64.max() - csq_min
    csq32 = csq64.astype(np.float32)

    v8f = v8.astype(np.float32)
    # per-value device-vs-true bound in band units: fp8 noise + fp16 half-ulp
    e_val = (E_MM + 0.5 * np.spacing(np.abs(v8))).astype(np.float32)
    # window: slots whose true max-alias score could plausibly win after csq
    W = csq_range * 1024.0 + SEL_NOISE + CHAIN_SLACK * 1024.0
    sel = (v8f[:, 0:1] - v8f) <= W        # [n, 8], always includes slot 0

    rr, ss = np.nonzero(sel)
    jj = i8[rr, ss].astype(np.int64)      # folded index in [0, K_FOLD)
    xs = x[rr]                            # [m, 512] f32
    score = np.empty((len(rr), N_ALIAS), dtype=np.float64)
    kk = np.empty((len(rr), N_ALIAS), dtype=np.int64)
    for a in range(N_ALIAS):
        ka = jj + a * K_FOLD
        kk[:, a] = ka
        score[:, a] = 2.0 * np.einsum("md,md->m", xs, codebook[ka]) - csq32[ka]

    # winner per row: max score, ties -> lowest k
    flat_r = np.repeat(rr, N_ALIAS)
    flat_s = score.reshape(-1)
    flat_k = kk.reshape(-1)
    order = np.lexsort((flat_k, -flat_s, flat_r))
    fr, fs, fk = flat_r[order], flat_s[order], flat_k[order]
    first = np.r_[True, fr[1:] != fr[:-1]]
    win_rows = fr[first]
    idx = np.zeros(n, dtype=np.int64)
    win_score = np.zeros(n, dtype=np.float64)
    runner = np.full(n, -np.inf)
    idx[win_rows] = fk[first]
    win_score[win_rows] = fs[first]
    pos = np.nonzero(first)[0]
    has2 = np.r_[pos[1:], len(fr)] - pos >= 2
    runner[win_rows[has2]] = fs[pos[has2] + 1]

    # flags (margin widened for the f32 resolve's own rounding)
    margin_flag = (win_score - runner) < MARGIN_THR
    hidden_ub = (v8f[:, 7] + e_val[:, 7] - np.float32(BAND_SHIFT)) * (2.0 ** -10) - csq_min
    hidden_flag = win_score < hidden_ub + CHAIN_SLACK
    dup_in_w = np.any((v8[:, :-1] == v8[:, 1:]) & sel[:, 1:], axis=1)
    flagged = np.nonzero(margin_flag | hidden_flag | dup_in_w)[0]
    return idx, flagged


def _exact_chain_rows(x, codebook, rows):
    """Reference's exact fp32 rounding chain for the given rows (f64 math)."""
    x64 = x[rows].astype(np.float64)
    cb64 = codebook.astype(np.float64)
    xsq32 = np.einsum("md,md->m", x64, x64).astype(np.float32)
    csq32 = np.einsum("kd,kd->k", cb64, cb64).astype(np.float32)
    cr32 = (2.0 * (x64 @ cb64.T)).astype(np.float32)
    d1 = (xsq32[:, None].astype(np.float64) - cr32.astype(np.float64)).astype(np.float32)
    d2 = (d1.astype(np.float64) + csq32.astype(np.float64)[None, :]).astype(np.float32)
    return np.argmin(d2, axis=1).astype(np.int64)


def kernel(x, codebook, embedding, **run_kwargs):
    x = np.ascontiguousarray(np.asarray(x, dtype=np.float32))
    codebook = np.ascontiguousarray(np.asarray(codebook, dtype=np.float32))
    embedding = np.ascontiguousarray(np.asarray(embedding, dtype=np.float32))
    n = x.shape[0]
    n_shard = n // N_CORES
    nc = _get_nc()

    xq8 = (x.T * np.float32(X_SCALE)).astype(FP8)         # [512, n]
    cq8 = (codebook.T * np.float32(C_SCALE)).astype(FP8)  # [512, 8192]
    cdr8 = _pack_dr(cq8)                                  # [256, 2*8192]
    xdr8_full = _pack_dr(xq8)                             # [256, 2*n]
    in_maps = []
    for i in range(N_CORES):
        sl = xdr8_full.reshape(2 * P, 2, n)[:, :, i * n_shard : (i + 1) * n_shard]
        in_maps.append(
            {
                "xdr8": np.ascontiguousarray(sl.reshape(2 * P, 2 * n_shard)),
                "cdr8": cdr8,
            }
        )
    res = run_bass_kernel_spmd(nc, in_maps, core_ids=list(range(N_CORES)), **run_kwargs)
    v8 = np.concatenate([res.results[i]["v8_out"] for i in range(N_CORES)], axis=0)
    i8 = np.concatenate([res.results[i]["i8_out"] for i in range(N_CORES)], axis=0)
    kernel.last_results = res

    idx, flagged = _host_decide(x, codebook, v8, i8)
    if flagged.size:
        idx[flagged] = _exact_chain_rows(x, codebook, flagged)
    kernel.n_flagged = len(flagged)
    return embedding[idx]


# revision 15
# speedup vs baseline: 2.6645x; 1.0233x over previous
"""Trainium2 Bass kernel for AudioQuantizer (VQ codebook lookup).

Computes, for x [N, 512], codebook [8192, 512], embedding [8192, 512]:
    dist[n,k] = ||x_n||^2 - 2 x_n.c_k + ||c_k||^2
    out[n]    = embedding[argmin_k dist[n,k]]
Sharding: data-parallel over N across 8 cores (codebook replicated).

Device side (per core, n_shard=4096): the PE computes only the cross term
    v[n,k] ~ 2^10 * (2 x_n.c_k)
as an fp8e4m3 DoubleRow matmul sweep: each 128x128 PE cell holds two fp8
weights, so one matmul covers a 256-deep contraction and the whole d=512
reduction takes 2 matmuls per 512-wide k-chunk (vs 4 at fp16) at ~1.8x the
fp16 column rate.  Operands are host-packed: lhsT plane i of partition p
holds x[d = dcp*256 + i*128 + p] (and likewise the codebook), matching the
[Ki, 2, dim] DoubleRow access pattern (validated bit-exact in CoreSim).
Weights stay stationary across the 8 k-chunks of each PSUM half-sweep.

The Act engine evacuates 7.5 of the 8 double-bank psum tiles per row-tile
into an fp16 band with a -1664 shift (top scores near 0, small fp16 ulp);
the DVE copies the remaining 512 so Act (the pacer) matches PE.  The DVE
then FOLDS the band three times with tensor_tensor(max) -- the only
reduction op with the 2x_1p 16-bit fast path -- so MAX8 + FIND_INDEX8
(1 elem/cycle, no 16-bit speedup) scan only 1024 of the 8192 scores.  A
folded slot j stands for the 8 candidates {j + 1024*a}.

Host side: fp8 quantization noise is bounded (measured max 111, budget 130
band units; 1 unit = 2^-10 in 2x.c terms), and c_sq spans only ~22 units,
so any slot within ~175 units of the top could win: those slots' 8 alias
candidates (~3 slots/row) are re-scored in f32 and the winner takes the
argmin tie rule.  Rows fall back to the reference's exact fp32 rounding
chain when the winner margin is under the chain+f32 slack, a duplicated
fp16 value makes FIND_INDEX8's first-occurrence indices unreliable, or the
8th folded value is close enough that a non-exported slot could hide a
contender.  ~1k rows flag; validated 0/32768 mismatches on the emulated
pipeline.

Startup: input DMAs are split across the SP and Activation DGE queues (they
serialize per queue), tile-0's weights and first k-chunks live in separate
head tiles, and the bulk codebook pieces are emitted interleaved with
tile-0's matmuls so the first matmul only waits on the head DMAs.

The walrus build here encodes at most one sync-wait per instruction, so
after Tile scheduling we hoist excess waits onto standalone EventSemaphore
instructions (split_multi_waits).
"""

from contextlib import ExitStack

import numpy as np
import ml_dtypes

import concourse.bass as bass
import concourse.mybir as mybir
import concourse.tile as tile
from concourse.bass_utils import run_bass_kernel_spmd

F32 = mybir.dt.float32
F16 = mybir.dt.float16
F8 = mybir.dt.float8e4
U32 = mybir.dt.uint32
FP8 = ml_dtypes.float8_e4m3  # IEEE e4m3 (max 240) -- matches mybir float8e4

P = 128
KC = 512           # k-chunk: psum free dim per matmul group
N_CORES = 8
N_TOTAL = 32768
K_TOTAL = 8192
D = 512

X_SCALE = 4.0      # fp8(x * 2^2): |x| <= ~5.5 -> 22
C_SCALE = 2048.0   # fp8(c * 2^11): |c| <= ~0.055 -> 112 (< 240 cap)
PSUM_SCALE = float(2.0 ** -2)  # 2^12*(2x.c) -> 2^10*(2x.c); no shift: fp8
# noise (~111 units) dwarfs the unshifted fp16 band ulp (<=2 units)
N_FOLD = 3                     # band 8192 -> 1024, 8 alias candidates/slot
K_FOLD = K_TOTAL >> N_FOLD


def split_multi_waits(nc, max_waits=1):
    """Hoist excess sync-waits onto standalone EventSemaphore instructions.

    The walrus build here rejects instructions carrying more than one
    sync-wait ("Too many sync wait commands").  Tile attaches several.
    An EventSemaphore on the same engine queue immediately before the
    instruction is semantically equivalent (the queue stalls there).
    """
    n_new = 0
    for f in nc.m.functions:
        for bb in f.blocks:
            insts = list(bb.instructions)
            out = []
            for inst in insts:
                si = inst.sync_info
                waits = list(si.on_wait) if si is not None and si.on_wait else []
                if len(waits) > max_waits:
                    keep = waits[-max_waits:]
                    for i, w in enumerate(waits[:-max_waits]):
                        ev = mybir.InstEventSemaphore(
                            name=f"{inst.name}_hw{i}", ins=[], outs=[]
                        )
                        ev.engine = inst.engine
                        ev.sync_info = mybir.SyncInfo(on_wait=[w], on_update=[])
                        out.append(ev)
                        n_new += 1
                    inst.sync_info = mybir.SyncInfo(
                        on_wait=keep, on_update=list(si.on_update or [])
                    )
                out.append(inst)
            if len(out) != len(insts):
                bb.instructions = out
    return n_new


def build_kernel(n_shard=N_TOTAL // N_CORES, k_total=K_TOTAL, d=D):
    nc = bass.Bass("TRN2", target_bir_lowering=False, debug=False)

    n_tiles = n_shard // P
    n_dcp = d // 256               # DoubleRow d-chunk pairs (contract 256 each)
    k_half = k_total // 2
    assert n_tiles * P == n_shard and n_dcp * 256 == d

    xt_ext = nc.dram_tensor("xdr8", [n_dcp * P, 2 * n_shard], F8, kind="ExternalInput").ap()
    cbt_ext = nc.dram_tensor("cdr8", [n_dcp * P, 2 * k_total], F8, kind="ExternalInput").ap()
    v8_ext = nc.dram_tensor("v8_out", [n_shard, 8], F16, kind="ExternalOutput").ap()
    i8_ext = nc.dram_tensor("i8_out", [n_shard, 8], U32, kind="ExternalOutput").ap()

    with tile.TileContext(nc) as tc, ExitStack() as ctx:
        consts = ctx.enter_context(tc.tile_pool(name="consts", bufs=1))
        v8a = consts.tile([P, n_tiles * 8], F16, name="v8a")
        i8a = consts.tile([P, n_tiles * 8], U32, name="i8a")

        xt_pool = ctx.enter_context(tc.tile_pool(name="xt", bufs=1))
        cb_pool = ctx.enter_context(tc.tile_pool(name="cb", bufs=1))
        # head tiles: tile-0 weights + k-chunks 0-1, so the first matmuls wait
        # only on these small DMAs (DMA-completion waits are cumulative per
        # queue).  packed column layout per partition: [plane0 | plane1].
        xh = [xt_pool.tile([P, 2 * P], F8, name=f"xh{q}") for q in range(n_dcp)]
        ch = [cb_pool.tile([P, 4 * KC], F8, name=f"ch{q}") for q in range(n_dcp)]
        xdr = [
            xt_pool.tile([P, 2 * n_shard], F8, name=f"xdr{q}") for q in range(n_dcp)
        ]
        cdr = [
            cb_pool.tile([P, 2 * k_total], F8, name=f"cdr{q}") for q in range(n_dcp)
        ]
        xhv = [t[:].rearrange("p (i n) -> p i n", i=2) for t in xh]
        chv = [t[:].rearrange("p (i n) -> p i n", i=2) for t in ch]
        xv = [t[:].rearrange("p (i n) -> p i n", i=2) for t in xdr]
        cv = [t[:].rearrange("p (i n) -> p i n", i=2) for t in cdr]

        # ---- head DMAs (split across the two hwdge queues: SP + Act) ----
        for q in range(n_dcp):
            rs = slice(q * P, (q + 1) * P)
            nc.sync.dma_start(xh[q][:, 0:P], xt_ext[rs, 0:P])
            nc.scalar.dma_start(xh[q][:, P : 2 * P], xt_ext[rs, n_shard : n_shard + P])
            nc.sync.dma_start(ch[q][:, 0 : 2 * KC], cbt_ext[rs, 0 : 2 * KC])
            nc.scalar.dma_start(
                ch[q][:, 2 * KC : 4 * KC], cbt_ext[rs, k_total : k_total + 2 * KC]
            )

        def cb_piece(c0, c1):  # k-chunks [c0, c1): both planes, both dcp
            for q in range(n_dcp):
                rs = slice(q * P, (q + 1) * P)
                nc.sync.dma_start(
                    cdr[q][:, c0 * KC : c1 * KC], cbt_ext[rs, c0 * KC : c1 * KC]
                )
                nc.scalar.dma_start(
                    cdr[q][:, k_total + c0 * KC : k_total + c1 * KC],
                    cbt_ext[rs, k_total + c0 * KC : k_total + c1 * KC],
                )

        def xt_piece(lo, hi):  # x columns [lo, hi): both planes, both dcp
            for q in range(n_dcp):
                rs = slice(q * P, (q + 1) * P)
                nc.sync.dma_start(xdr[q][:, lo:hi], xt_ext[rs, lo:hi])
                nc.scalar.dma_start(
                    xdr[q][:, n_shard + lo : n_shard + hi],
                    xt_ext[rs, n_shard + lo : n_shard + hi],
                )

        band_pool = ctx.enter_context(tc.tile_pool(name="band", bufs=2))
        fold_pool = ctx.enter_context(tc.tile_pool(name="fold", bufs=2))
        mm_psum = ctx.enter_context(tc.tile_pool(name="mmps", bufs=4, space="PSUM"))

        for t in range(n_tiles):
            band = band_pool.tile([P, 14 * KC], F16, tag="band")
            t1 = fold_pool.tile([P, k_half], F16, tag="t1")
            for h in range(2):
                pst = [
                    mm_psum.tile([P, 2 * KC], F32, tag="mm", name=f"mm{q}")
                    for q in range(4)
                ]
                for dcp in range(n_dcp):
                    for c in range(8):
                        kc = h * 8 + c
                        # bulk codebook pieces must be EMITTED before their
                        # first reader (program order defines RAW deps), but
                        # after the head-chunk matmuls so those only wait on
                        # the head DMAs.
                        if t == 0 and h == 0 and dcp == 0 and c == 2:
                            cb_piece(2, 8)
                        if t == 0 and kc >= 2:
                            lhs = xhv[dcp][:, :, 0:P]
                            rhs = cv[dcp][:, :, kc * KC : (kc + 1) * KC]
                        elif t == 0:
                            lhs = xhv[dcp][:, :, 0:P]
                            rhs = chv[dcp][:, :, kc * KC : (kc + 1) * KC]
                        else:
                            lhs = xv[dcp][:, :, t * P : (t + 1) * P]
                            rhs = (
                                chv[dcp][:, :, kc * KC : (kc + 1) * KC]
                                if kc < 2
                                else cv[dcp][:, :, kc * KC : (kc + 1) * KC]
                            )
                        nc.tensor.matmul(
                            pst[c // 2][:, (c % 2) * KC : (c % 2 + 1) * KC],
                            lhs,
                            rhs,
                            start=(dcp == 0),
                            stop=(dcp == n_dcp - 1),
                            perf_mode=mybir.MatmulPerfMode.DoubleRow,
                            skip_group_check=True,
                        )
                for q in range(4):
                    k0 = h * 8 * KC + q * 2 * KC
                    if h == 1 and q == 3:
                        # chunks 14-15: DVE folds psum straight into t1
                        # (k j+4096 vs band k j for j in [3072, 4096))
                        nc.vector.scalar_tensor_tensor(
                            t1[:, 3072:4096],
                            pst[q][:],
                            float(PSUM_SCALE),
                            band[:, 3072:4096],
                            op0=mybir.AluOpType.mult,
                            op1=mybir.AluOpType.max,
                        )
                    else:
                        nc.scalar.mul(band[:, k0 : k0 + 2 * KC], pst[q][:], PSUM_SCALE)
                if t == 0 and h == 0:
                    cb_piece(8, 16)
            if t == 0:
                xt_piece(P, n_shard)

            nc.vector.tensor_tensor(
                out=t1[:, 0:3072],
                in0=band[:, 0:3072],
                in1=band[:, 4096:7168],
                op=mybir.AluOpType.max,
            )
            t2 = fold_pool.tile([P, k_half // 2], F16, tag="t2")
            nc.vector.tensor_tensor(
                out=t2[:],
                in0=t1[:, 0 : k_half // 2],
                in1=t1[:, k_half // 2 : k_half],
                op=mybir.AluOpType.max,
            )
            t3 = fold_pool.tile([P, K_FOLD], F16, tag="t3")
            nc.vector.tensor_tensor(
                out=t3[:],
                in0=t2[:, 0:K_FOLD],
                in1=t2[:, K_FOLD : k_half // 2],
                op=mybir.AluOpType.max,
            )
            v8s = v8a[:, t * 8 : (t + 1) * 8]
            nc.vector.max(v8s, t3[:])
            nc.vector.max_index(i8a[:, t * 8 : (t + 1) * 8], v8s, t3[:])

        nc.sync.dma_start(
            v8_ext.rearrange("(t p) e -> p t e", p=P),
            v8a[:].rearrange("p (t e) -> p t e", e=8),
        )
        nc.sync.dma_start(
            i8_ext.rearrange("(t p) e -> p t e", p=P),
            i8a[:].rearrange("p (t e) -> p t e", e=8),
        )

    return nc


_NC_CACHE = {}


def _get_nc():
    if "nc" not in _NC_CACHE:
        nc = build_kernel()
        split_multi_waits(nc)
        _NC_CACHE["nc"] = nc
    return _NC_CACHE["nc"]


def _pack_dr(arrT):
    """[d, cols] -> DoubleRow-packed [n_dcp*128, 2*cols] (plane-major)."""
    d = arrT.shape[0]
    out = []
    for dcp in range(d // 256):
        pl = arrT[dcp * 256 : (dcp + 1) * 256]          # [256, cols]
        out.append(
            np.ascontiguousarray(
                np.stack([pl[0:P], pl[P : 2 * P]], axis=1).reshape(P, -1)
            )
        )
    return np.concatenate(out, axis=0)


# ---------------- host side ----------------

# band-unit error budget (1 unit = 2^-10 raw 2x.c):
E_MM = 130.0        # fp8 matmul quantization noise hard ceiling (measured max 111)
SEL_NOISE = 150.0   # selection-window noise allowance (~6 sigma of error diff)
CHAIN_SLACK = 3e-4  # reference fp32 rounding-chain slack, raw units
MARGIN_THR = 4e-4   # raw-unit winner margin below which we replay the chain
N_ALIAS = 1 << N_FOLD


def _host_decide(x, codebook, v8, i8):
    """Resolve folded top-8 candidates; return (idx, flagged_rows)."""
    n = x.shape[0]
    cb64 = codebook.astype(np.float64)
    csq64 = np.einsum("kd,kd->k", cb64, cb64)
    csq_min = csq64.min()
    csq_range = csq64.max() - csq_min
    csq32 = csq64.astype(np.float32)

    v8f = v8.astype(np.float32)
    # per-value device-vs-true bound in band units: fp8 noise + fp16 half-ulp
    e_val = (E_MM + 0.5 * np.spacing(np.abs(v8))).astype(np.float32)
    # window: slots whose true max-alias score could plausibly win after csq
    W = csq_range * 1024.0 + SEL_NOISE + CHAIN_SLACK * 1024.0
    sel = (v8f[:, 0:1] - v8f) <= W        # [n, 8], always includes slot 0

    rr, ss = np.nonzero(sel)
    jj = i8[rr, ss].astype(np.int64)      # folded index in [0, K_FOLD)
    xs = x[rr]                            # [m, 512] f32
    score = np.empty((len(rr), N_ALIAS), dtype=np.float64)
    kk = np.empty((len(rr), N_ALIAS), dtype=np.int64)
    for a in range(N_ALIAS):
        ka = jj + a * K_FOLD
        kk[:, a] = ka
        score[:, a] = 2.0 * np.einsum("md,md->m", xs, codebook[ka]) - csq32[ka]

    # winner per row: max score, ties -> lowest k
    flat_r = np.repeat(rr, N_ALIAS)
    flat_s = score.reshape(-1)
    flat_k = kk.reshape(-1)
    order = np.lexsort((flat_k, -flat_s, flat_r))
    fr, fs, fk = flat_r[order], flat_s[order], flat_k[order]
    first = np.r_[True, fr[1:] != fr[:-1]]
    win_rows = fr[first]
    idx = np.zeros(n, dtype=np.int64)
    win_score = np.zeros(n, dtype=np.float64)
    runner = np.full(n, -np.inf)
    idx[win_rows] = fk[first]
    win_score[win_rows] = fs[first]
    pos = np.nonzero(first)[0]
    has2 = np.r_[pos[1:], len(fr)] - pos >= 2
    runner[win_rows[has2]] = fs[pos[has2] + 1]

    # flags (margin widened for the f32 resolve's own rounding)
    margin_flag = (win_score - runner) < MARGIN_THR
    hidden_ub = (v8f[:, 7] + e_val[:, 7]) * (2.0 ** -10) - csq_min
    hidden_flag = win_score < hidden_ub + CHAIN_SLACK
    dup_in_w = np.any((v8[:, :-1] == v8[:, 1:]) & sel[:, 1:], axis=1)
    flagged = np.nonzero(margin_flag | hidden_flag | dup_in_w)[0]
    return idx, flagged


def _exact_chain_rows(x, codebook, rows):
    """Reference's exact fp32 rounding chain for the given rows (f64 math)."""
    x64 = x[rows].astype(np.float64)
    cb64 = codebook.astype(np.float64)
    xsq32 = np.einsum("md,md->m", x64, x64).astype(np.float32)
    csq32 = np.einsum("kd,kd->k", cb64, cb64).astype(np.float32)
    cr32 = (2.0 * (x64 @ cb64.T)).astype(np.float32)
    d1 = (xsq32[:, None].astype(np.float64) - cr32.astype(np.float64)).astype(np.float32)
    d2 = (d1.astype(np.float64) + csq32.astype(np.float64)[None, :]).astype(np.float32)
    return np.argmin(d2, axis=1).astype(np.int64)


def kernel(x, codebook, embedding, **run_kwargs):
    x = np.ascontiguousarray(np.asarray(x, dtype=np.float32))
    codebook = np.ascontiguousarray(np.asarray(codebook, dtype=np.float32))
    embedding = np.ascontiguousarray(np.asarray(embedding, dtype=np.float32))
    n = x.shape[0]
    n_shard = n // N_CORES
    nc = _get_nc()

    xq8 = (x.T * np.float32(X_SCALE)).astype(FP8)         # [512, n]
    cq8 = (codebook.T * np.float32(C_SCALE)).astype(FP8)  # [512, 8192]
    cdr8 = _pack_dr(cq8)                                  # [256, 2*8192]
    xdr8_full = _pack_dr(xq8)                             # [256, 2*n]
    in_maps = []
    for i in range(N_CORES):
        sl = xdr8_full.reshape(2 * P, 2, n)[:, :, i * n_shard : (i + 1) * n_shard]
        in_maps.append(
            {
                "xdr8": np.ascontiguousarray(sl.reshape(2 * P, 2 * n_shard)),
                "cdr8": cdr8,
            }
        )
    res = run_bass_kernel_spmd(nc, in_maps, core_ids=list(range(N_CORES)), **run_kwargs)
    v8 = np.concatenate([res.results[i]["v8_out"] for i in range(N_CORES)], axis=0)
    i8 = np.concatenate([res.results[i]["i8_out"] for i in range(N_CORES)], axis=0)
    kernel.last_results = res

    idx, flagged = _host_decide(x, codebook, v8, i8)
    if flagged.size:
        idx[flagged] = _exact_chain_rows(x, codebook, flagged)
    kernel.n_flagged = len(flagged)
    return embedding[idx]


# revision 16
# speedup vs baseline: 2.7519x; 1.0328x over previous
"""Trainium2 Bass kernel for AudioQuantizer (VQ codebook lookup).

Computes, for x [N, 512], codebook [8192, 512], embedding [8192, 512]:
    dist[n,k] = ||x_n||^2 - 2 x_n.c_k + ||c_k||^2
    out[n]    = embedding[argmin_k dist[n,k]]
Sharding: data-parallel over N across 8 cores (codebook replicated).

Device side (per core, n_shard=4096): the PE computes only the cross term
    v[n,k] ~ 2^10 * (2 x_n.c_k)
as an fp8e4m3 DoubleRow matmul sweep: each 128x128 PE cell holds two fp8
weights, so one matmul covers a 256-deep contraction and the whole d=512
reduction takes 2 matmuls per 512-wide k-chunk (vs 4 at fp16) at ~1.8x the
fp16 column rate.  Operands are host-packed: lhsT plane i of partition p
holds x[d = dcp*256 + i*128 + p] (and likewise the codebook), matching the
[Ki, 2, dim] DoubleRow access pattern (validated bit-exact in CoreSim).
Weights stay stationary across the 8 k-chunks of each PSUM half-sweep.

The Act engine evacuates 7.5 of the 8 double-bank psum tiles per row-tile
into an fp16 band with a -1664 shift (top scores near 0, small fp16 ulp);
the DVE copies the remaining 512 so Act (the pacer) matches PE.  The DVE
then FOLDS the band three times with tensor_tensor(max) -- the only
reduction op with the 2x_1p 16-bit fast path -- so MAX8 + FIND_INDEX8
(1 elem/cycle, no 16-bit speedup) scan only 1024 of the 8192 scores.  A
folded slot j stands for the 8 candidates {j + 1024*a}.

Host side: fp8 quantization noise is bounded (measured max 111, budget 130
band units; 1 unit = 2^-10 in 2x.c terms), and c_sq spans only ~22 units,
so any slot within ~175 units of the top could win: those slots' 8 alias
candidates (~3 slots/row) are re-scored in f32 and the winner takes the
argmin tie rule.  Rows fall back to the reference's exact fp32 rounding
chain when the winner margin is under the chain+f32 slack, a duplicated
fp16 value makes FIND_INDEX8's first-occurrence indices unreliable, or the
8th folded value is close enough that a non-exported slot could hide a
contender.  ~1k rows flag; validated 0/32768 mismatches on the emulated
pipeline.

Startup: input DMAs are split across the SP and Activation DGE queues (they
serialize per queue), tile-0's weights and first k-chunks live in separate
head tiles, and the bulk codebook pieces are emitted interleaved with
tile-0's matmuls so the first matmul only waits on the head DMAs.

The walrus build here encodes at most one sync-wait per instruction, so
after Tile scheduling we hoist excess waits onto standalone EventSemaphore
instructions (split_multi_waits).
"""

from contextlib import ExitStack

import numpy as np
import ml_dtypes

import concourse.bass as bass
import concourse.mybir as mybir
import concourse.tile as tile
from concourse.bass_utils import run_bass_kernel_spmd

F32 = mybir.dt.float32
F16 = mybir.dt.float16
F8 = mybir.dt.float8e4
U32 = mybir.dt.uint32
FP8 = ml_dtypes.float8_e4m3  # IEEE e4m3 (max 240) -- matches mybir float8e4

P = 128
KC = 512           # k-chunk: psum free dim per matmul group
N_CORES = 8
N_TOTAL = 32768
K_TOTAL = 8192
D = 512

X_SCALE = 4.0      # fp8(x * 2^2): |x| <= ~5.5 -> 22
C_SCALE = 2048.0   # fp8(c * 2^11): |c| <= ~0.055 -> 112 (< 240 cap)
PSUM_SCALE = float(2.0 ** -2)  # 2^12*(2x.c) -> 2^10*(2x.c); no shift: fp8
# noise (~111 units) dwarfs the unshifted fp16 band ulp (<=2 units)
N_FOLD = 3                     # band 8192 -> 1024, 8 alias candidates/slot
K_FOLD = K_TOTAL >> N_FOLD


def split_multi_waits(nc, max_waits=1):
    """Hoist excess sync-waits onto standalone EventSemaphore instructions.

    The walrus build here rejects instructions carrying more than one
    sync-wait ("Too many sync wait commands").  Tile attaches several.
    An EventSemaphore on the same engine queue immediately before the
    instruction is semantically equivalent (the queue stalls there).
    """
    n_new = 0
    for f in nc.m.functions:
        for bb in f.blocks:
            insts = list(bb.instructions)
            out = []
            for inst in insts:
                si = inst.sync_info
                waits = list(si.on_wait) if si is not None and si.on_wait else []
                if len(waits) > max_waits:
                    keep = waits[-max_waits:]
                    for i, w in enumerate(waits[:-max_waits]):
                        ev = mybir.InstEventSemaphore(
                            name=f"{inst.name}_hw{i}", ins=[], outs=[]
                        )
                        ev.engine = inst.engine
                        ev.sync_info = mybir.SyncInfo(on_wait=[w], on_update=[])
                        out.append(ev)
                        n_new += 1
                    inst.sync_info = mybir.SyncInfo(
                        on_wait=keep, on_update=list(si.on_update or [])
                    )
                out.append(inst)
            if len(out) != len(insts):
                bb.instructions = out
    return n_new


def build_kernel(n_shard=N_TOTAL // N_CORES, k_total=K_TOTAL, d=D):
    nc = bass.Bass("TRN2", target_bir_lowering=False, debug=False)

    n_tiles = n_shard // P
    n_dcp = d // 256               # DoubleRow d-chunk pairs (contract 256 each)
    k_half = k_total // 2
    assert n_tiles * P == n_shard and n_dcp * 256 == d

    xt_ext = nc.dram_tensor("xdr8", [n_dcp * P, 2 * n_shard], F8, kind="ExternalInput").ap()
    cbt_ext = nc.dram_tensor("cdr8", [n_dcp * P, 2 * k_total], F8, kind="ExternalInput").ap()
    # device-native layout [P, n_tiles*8] (contiguous DMA; host reshapes)
    v8_ext = nc.dram_tensor("v8_out", [P, (n_shard // P) * 8], F16, kind="ExternalOutput").ap()
    i8_ext = nc.dram_tensor("i8_out", [P, (n_shard // P) * 8], U32, kind="ExternalOutput").ap()

    with tile.TileContext(nc) as tc, ExitStack() as ctx:
        consts = ctx.enter_context(tc.tile_pool(name="consts", bufs=1))
        v8a = consts.tile([P, n_tiles * 8], F16, name="v8a")
        i8a = consts.tile([P, n_tiles * 8], U32, name="i8a")

        xt_pool = ctx.enter_context(tc.tile_pool(name="xt", bufs=1))
        cb_pool = ctx.enter_context(tc.tile_pool(name="cb", bufs=1))
        # head tiles: tile-0 weights + k-chunks 0-1, so the first matmuls wait
        # only on these small DMAs (DMA-completion waits are cumulative per
        # queue).  packed column layout per partition: [plane0 | plane1].
        xh = [xt_pool.tile([P, 2 * P], F8, name=f"xh{q}") for q in range(n_dcp)]
        ch = [cb_pool.tile([P, 4 * KC], F8, name=f"ch{q}") for q in range(n_dcp)]
        xdr = [
            xt_pool.tile([P, 2 * n_shard], F8, name=f"xdr{q}") for q in range(n_dcp)
        ]
        cdr = [
            cb_pool.tile([P, 2 * k_total], F8, name=f"cdr{q}") for q in range(n_dcp)
        ]
        xhv = [t[:].rearrange("p (i n) -> p i n", i=2) for t in xh]
        chv = [t[:].rearrange("p (i n) -> p i n", i=2) for t in ch]
        xv = [t[:].rearrange("p (i n) -> p i n", i=2) for t in xdr]
        cv = [t[:].rearrange("p (i n) -> p i n", i=2) for t in cdr]

        # ---- head DMAs (split across the two hwdge queues: SP + Act) ----
        for q in range(n_dcp):
            rs = slice(q * P, (q + 1) * P)
            nc.sync.dma_start(xh[q][:, 0:P], xt_ext[rs, 0:P])
            nc.scalar.dma_start(xh[q][:, P : 2 * P], xt_ext[rs, n_shard : n_shard + P])
            nc.sync.dma_start(ch[q][:, 0 : 2 * KC], cbt_ext[rs, 0 : 2 * KC])
            nc.scalar.dma_start(
                ch[q][:, 2 * KC : 4 * KC], cbt_ext[rs, k_total : k_total + 2 * KC]
            )

        def cb_piece(c0, c1):  # k-chunks [c0, c1): both planes, both dcp
            for q in range(n_dcp):
                rs = slice(q * P, (q + 1) * P)
                nc.sync.dma_start(
                    cdr[q][:, c0 * KC : c1 * KC], cbt_ext[rs, c0 * KC : c1 * KC]
                )
                nc.scalar.dma_start(
                    cdr[q][:, k_total + c0 * KC : k_total + c1 * KC],
                    cbt_ext[rs, k_total + c0 * KC : k_total + c1 * KC],
                )

        def xt_piece(lo, hi):  # x columns [lo, hi): both planes, both dcp
            for q in range(n_dcp):
                rs = slice(q * P, (q + 1) * P)
                nc.sync.dma_start(xdr[q][:, lo:hi], xt_ext[rs, lo:hi])
                nc.scalar.dma_start(
                    xdr[q][:, n_shard + lo : n_shard + hi],
                    xt_ext[rs, n_shard + lo : n_shard + hi],
                )

        band_pool = ctx.enter_context(tc.tile_pool(name="band", bufs=2))
        fold_pool = ctx.enter_context(tc.tile_pool(name="fold", bufs=2))
        mm_psum = ctx.enter_context(tc.tile_pool(name="mmps", bufs=4, space="PSUM"))

        for t in range(n_tiles):
            band = band_pool.tile([P, 14 * KC], F16, tag="band")
            t1 = fold_pool.tile([P, k_half], F16, tag="t1")
            for h in range(2):
                pst = [
                    mm_psum.tile([P, 2 * KC], F32, tag="mm", name=f"mm{q}")
                    for q in range(4)
                ]
                for dcp in range(n_dcp):
                    for c in range(8):
                        kc = h * 8 + c
                        # bulk codebook pieces must be EMITTED before their
                        # first reader (program order defines RAW deps), but
                        # after the head-chunk matmuls so those only wait on
                        # the head DMAs.
                        if t == 0 and h == 0 and dcp == 0 and c == 2:
                            cb_piece(2, 8)
                        if t == 0 and kc >= 2:
                            lhs = xhv[dcp][:, :, 0:P]
                            rhs = cv[dcp][:, :, kc * KC : (kc + 1) * KC]
                        elif t == 0:
                            lhs = xhv[dcp][:, :, 0:P]
                            rhs = chv[dcp][:, :, kc * KC : (kc + 1) * KC]
                        else:
                            lhs = xv[dcp][:, :, t * P : (t + 1) * P]
                            rhs = (
                                chv[dcp][:, :, kc * KC : (kc + 1) * KC]
                                if kc < 2
                                else cv[dcp][:, :, kc * KC : (kc + 1) * KC]
                            )
                        nc.tensor.matmul(
                            pst[c // 2][:, (c % 2) * KC : (c % 2 + 1) * KC],
                            lhs,
                            rhs,
                            start=(dcp == 0),
                            stop=(dcp == n_dcp - 1),
                            perf_mode=mybir.MatmulPerfMode.DoubleRow,
                            skip_group_check=True,
                        )
                for q in range(4):
                    k0 = h * 8 * KC + q * 2 * KC
                    if h == 1 and q == 3:
                        # chunks 14-15: DVE folds psum straight into t1
                        # (k j+4096 vs band k j for j in [3072, 4096))
                        nc.vector.scalar_tensor_tensor(
                            t1[:, 3072:4096],
                            pst[q][:],
                            float(PSUM_SCALE),
                            band[:, 3072:4096],
                            op0=mybir.AluOpType.mult,
                            op1=mybir.AluOpType.max,
                        )
                    else:
                        nc.scalar.mul(band[:, k0 : k0 + 2 * KC], pst[q][:], PSUM_SCALE)
                if t == 0 and h == 0:
                    cb_piece(8, 16)
            if t == 0:
                xt_piece(P, n_shard)

            nc.vector.tensor_tensor(
                out=t1[:, 0:3072],
                in0=band[:, 0:3072],
                in1=band[:, 4096:7168],
                op=mybir.AluOpType.max,
            )
            t2 = fold_pool.tile([P, k_half // 2], F16, tag="t2")
            nc.vector.tensor_tensor(
                out=t2[:],
                in0=t1[:, 0 : k_half // 2],
                in1=t1[:, k_half // 2 : k_half],
                op=mybir.AluOpType.max,
            )
            t3 = fold_pool.tile([P, K_FOLD], F16, tag="t3")
            nc.vector.tensor_tensor(
                out=t3[:],
                in0=t2[:, 0:K_FOLD],
                in1=t2[:, K_FOLD : k_half // 2],
                op=mybir.AluOpType.max,
            )
            v8s = v8a[:, t * 8 : (t + 1) * 8]
            nc.vector.max(v8s, t3[:])
            nc.vector.max_index(i8a[:, t * 8 : (t + 1) * 8], v8s, t3[:])

        nc.sync.dma_start(v8_ext, v8a[:])
        nc.sync.dma_start(i8_ext, i8a[:])

    return nc


_NC_CACHE = {}


def _get_nc():
    if "nc" not in _NC_CACHE:
        nc = build_kernel()
        split_multi_waits(nc)
        _NC_CACHE["nc"] = nc
    return _NC_CACHE["nc"]


def _pack_dr(arrT):
    """[d, cols] -> DoubleRow-packed [n_dcp*128, 2*cols] (plane-major)."""
    d = arrT.shape[0]
    out = []
    for dcp in range(d // 256):
        pl = arrT[dcp * 256 : (dcp + 1) * 256]          # [256, cols]
        out.append(
            np.ascontiguousarray(
                np.stack([pl[0:P], pl[P : 2 * P]], axis=1).reshape(P, -1)
            )
        )
    return np.concatenate(out, axis=0)


# ---------------- host side ----------------

# band-unit error budget (1 unit = 2^-10 raw 2x.c):
E_MM = 130.0        # fp8 matmul quantization noise hard ceiling (measured max 111)
SEL_NOISE = 150.0   # selection-window noise allowance (~6 sigma of error diff)
CHAIN_SLACK = 3e-4  # reference fp32 rounding-chain slack, raw units
MARGIN_THR = 4e-4   # raw-unit winner margin below which we replay the chain
N_ALIAS = 1 << N_FOLD


def _host_decide(x, codebook, v8, i8):
    """Resolve folded top-8 candidates; return (idx, flagged_rows)."""
    n = x.shape[0]
    cb64 = codebook.astype(np.float64)
    csq64 = np.einsum("kd,kd->k", cb64, cb64)
    csq_min = csq64.min()
    csq_range = csq64.max() - csq_min
    csq32 = csq64.astype(np.float32)

    v8f = v8.astype(np.float32)
    # per-value device-vs-true bound in band units: fp8 noise + fp16 half-ulp
    e_val = (E_MM + 0.5 * np.spacing(np.abs(v8))).astype(np.float32)
    # window: slots whose true max-alias score could plausibly win after csq
    W = csq_range * 1024.0 + SEL_NOISE + CHAIN_SLACK * 1024.0
    sel = (v8f[:, 0:1] - v8f) <= W        # [n, 8], always includes slot 0

    rr, ss = np.nonzero(sel)
    jj = i8[rr, ss].astype(np.int64)      # folded index in [0, K_FOLD)
    xs = x[rr]                            # [m, 512] f32
    score = np.empty((len(rr), N_ALIAS), dtype=np.float64)
    kk = np.empty((len(rr), N_ALIAS), dtype=np.int64)
    for a in range(N_ALIAS):
        ka = jj + a * K_FOLD
        kk[:, a] = ka
        score[:, a] = 2.0 * np.einsum("md,md->m", xs, codebook[ka]) - csq32[ka]

    # winner per row: max score, ties -> lowest k
    flat_r = np.repeat(rr, N_ALIAS)
    flat_s = score.reshape(-1)
    flat_k = kk.reshape(-1)
    order = np.lexsort((flat_k, -flat_s, flat_r))
    fr, fs, fk = flat_r[order], flat_s[order], flat_k[order]
    first = np.r_[True, fr[1:] != fr[:-1]]
    win_rows = fr[first]
    idx = np.zeros(n, dtype=np.int64)
    win_score = np.zeros(n, dtype=np.float64)
    runner = np.full(n, -np.inf)
    idx[win_rows] = fk[first]
    win_score[win_rows] = fs[first]
    pos = np.nonzero(first)[0]
    has2 = np.r_[pos[1:], len(fr)] - pos >= 2
    runner[win_rows[has2]] = fs[pos[has2] + 1]

    # flags (margin widened for the f32 resolve's own rounding)
    margin_flag = (win_score - runner) < MARGIN_THR
    hidden_ub = (v8f[:, 7] + e_val[:, 7]) * (2.0 ** -10) - csq_min
    hidden_flag = win_score < hidden_ub + CHAIN_SLACK
    dup_in_w = np.any((v8[:, :-1] == v8[:, 1:]) & sel[:, 1:], axis=1)
    flagged = np.nonzero(margin_flag | hidden_flag | dup_in_w)[0]
    return idx, flagged


def _exact_chain_rows(x, codebook, rows):
    """Reference's exact fp32 rounding chain for the given rows (f64 math)."""
    x64 = x[rows].astype(np.float64)
    cb64 = codebook.astype(np.float64)
    xsq32 = np.einsum("md,md->m", x64, x64).astype(np.float32)
    csq32 = np.einsum("kd,kd->k", cb64, cb64).astype(np.float32)
    cr32 = (2.0 * (x64 @ cb64.T)).astype(np.float32)
    d1 = (xsq32[:, None].astype(np.float64) - cr32.astype(np.float64)).astype(np.float32)
    d2 = (d1.astype(np.float64) + csq32.astype(np.float64)[None, :]).astype(np.float32)
    return np.argmin(d2, axis=1).astype(np.int64)


def kernel(x, codebook, embedding, **run_kwargs):
    x = np.ascontiguousarray(np.asarray(x, dtype=np.float32))
    codebook = np.ascontiguousarray(np.asarray(codebook, dtype=np.float32))
    embedding = np.ascontiguousarray(np.asarray(embedding, dtype=np.float32))
    n = x.shape[0]
    n_shard = n // N_CORES
    nc = _get_nc()

    xq8 = (x.T * np.float32(X_SCALE)).astype(FP8)         # [512, n]
    cq8 = (codebook.T * np.float32(C_SCALE)).astype(FP8)  # [512, 8192]
    cdr8 = _pack_dr(cq8)                                  # [256, 2*8192]
    xdr8_full = _pack_dr(xq8)                             # [256, 2*n]
    in_maps = []
    for i in range(N_CORES):
        sl = xdr8_full.reshape(2 * P, 2, n)[:, :, i * n_shard : (i + 1) * n_shard]
        in_maps.append(
            {
                "xdr8": np.ascontiguousarray(sl.reshape(2 * P, 2 * n_shard)),
                "cdr8": cdr8,
            }
        )
    res = run_bass_kernel_spmd(nc, in_maps, core_ids=list(range(N_CORES)), **run_kwargs)

    def unpack(name, dt):
        # [P, n_tiles*8] device layout -> [n_shard, 8]: row = t*P + p
        return np.concatenate(
            [
                np.ascontiguousarray(
                    res.results[i][name]
                    .reshape(P, n_shard // P, 8)
                    .transpose(1, 0, 2)
                ).reshape(n_shard, 8)
                for i in range(N_CORES)
            ],
            axis=0,
        )

    v8 = unpack("v8_out", np.float16)
    i8 = unpack("i8_out", np.uint32)
    kernel.last_results = res

    idx, flagged = _host_decide(x, codebook, v8, i8)
    if flagged.size:
        idx[flagged] = _exact_chain_rows(x, codebook, flagged)
    kernel.n_flagged = len(flagged)
    return embedding[idx]
